# revision 45
# baseline (speedup 1.0000x reference)
"""Trainium2 Bass kernel for point-cloud GRU (kNN set-conv gates, InstanceNorm).

Wall time through the axon tunnel is dominated by host<->device transfer
and per-call dispatch, not device compute, so the design minimizes moved
bytes and per-call work:
  - 4 cores per batch (B=2), each owning a 1024-point shard of S=4096.
  - Each core receives ONLY its own f16 h/x shard (tile-interleaved),
    a 1/8 slice of the f16 weight tables, and tiny f32 packs; ~0.9MB/core
    vs ~7.3MB/core for replicated-f32 inputs.
  - The per-point projection tables (set-conv linearization, below) are
    computed per-shard from the core's own data, then AllGathered (f16)
    within the batch group -- cheaper and shorter-chained than gathering
    raw h/x and computing full tables on every core.
  - Output is the f16 GRU delta z*(q-h) only; full-precision h is added
    back on the host.
  - A persistent XLA compilation cache is enabled: run_bass_kernel_spmd
    re-lowers+re-compiles its jit closure every call otherwise (~0.2s).

Device algorithm:
  - kNN (k=4): PE computes score[i,j] = |x_j|^2 - 2 x_i.x_j for own rows,
    DVE max8+max_index on negated scores -> 4 smallest (self included).
  - Set-conv is linearized: y[s,k,o] = w[idx[s,k], o] + c[o, s] where
    w[n,o] = W_feat.f[n] + W_xyz.xyz[n] (per-point projection table) and
    c[o,s] = b[o] - W_xyz.xyz[s].  Table rows gathered by neighbor index
    (SWDGE indirect DMA) from the AllGathered [S, 384] f16 table.
  - InstanceNorm stats over (S,k) per (b,o) from algebraic identities:
      sum y   = A + k*Cs,   A  = sum_s t[s],  t = sum_k w[idx[s,k]]
      sum y^2 = B2 + 2*X + k*C2,  B2 = sum_s sum_k w^2,  X = sum_s c.t
    A/B2/X via PE ones-matmuls; Cs/C2 via ScalarE accum; partials
    AllReduced across the 4-core batch group (tiny).
  - max_k commutes with the (monotonic) normalization: out uses m = max_k w.
  - Phase 2 (q gate) needs r at neighbor points: per-shard q-table tiles
    Wq_h.(r*h) are computed locally then AllGathered (f16, [S, 128]).
"""

import numpy as np

B, S, H, D = 2, 4096, 128, 256
O = 128
K = 4
NCORES = 8
GROUP = 4              # cores per batch
PTS = S // GROUP       # points per core
NT = S // 128          # 32 table M-tiles
ST = PTS // 128        # 8 own s-tiles
EPS = 1e-5
NK = float(S * K)
WCOLS = 3 * (3 * O) + O        # wt0|wt1|wt2|wqh stacked -> 1280 f16 cols
WSH = WCOLS // 8               # 1280 / 8 = 160
HXC = 3 * PTS          # 3072 f16 cols: per-tile [h|x0|x1] interleave

_CACHE = {}


def _build_program():
    from concourse import bass, bacc, mybir, tile
    from concourse.masks import make_identity

    dt = mybir.dt
    f32, f16, u32 = dt.float32, dt.float16, dt.uint32
    AF = mybir.ActivationFunctionType
    ALU = mybir.AluOpType

    nc = bacc.Bacc("TRN2", target_bir_lowering=False, debug=False,
                   enable_asserts=False, num_devices=NCORES)

    # ---------------- I/O ----------------
    # hx16 cols: 0:HXC tile-interleaved [h|x0|x1] shard, HXC:HXC+WSH weight slice
    hx16 = nc.dram_tensor("hx16", [128, HXC + WSH], f16,
                          kind="ExternalInput").ap()
    # ppack rows: 0:4 pca_my ([pc;sq] own shard), 4:8 pct ([-2pc;ones]);
    # cols PTS:PTS+3*O of rows 0:4 hold wtg (3 rows) + brow (row 3)
    ppack = nc.dram_tensor("ppack", [8, PTS + 3 * O], f32,
                           kind="ExternalInput").ap()
    bcol = nc.dram_tensor("bcol", [128, 3], f32, kind="ExternalInput").ap()
    # 12-bit packed delta: low byte [O, PTS] + two 4-bit highs per byte
    # [O, PTS/2], plus per-channel scale. 25% fewer output bytes than f16.
    out_io = nc.dram_tensor("out", [O, PTS + PTS // 2], dt.int8,
                            kind="ExternalOutput").ap()
    osc_io = nc.dram_tensor("osc", [O, 1], f32, kind="ExternalOutput").ap()

    # ---------------- internal DRAM ----------------
    ag0_in = nc.dram_tensor("ag0_in", [4, PTS], f32, kind="Internal").ap()
    ag0_out = nc.dram_tensor("ag0_out", [GROUP, 4, PTS], f32,
                             kind="Internal").ap()
    ag2_in = nc.dram_tensor("ag2_in", [128, WSH], f16, kind="Internal").ap()
    ag2_out = nc.dram_tensor("ag2_out", [NCORES, 128, WSH], f16,
                             kind="Internal").ap()
    tb1own = nc.dram_tensor("tb1own", [PTS, 3 * O], f16, kind="Internal").ap()
    tb1 = nc.dram_tensor("tb1", [S, 3 * O], f16, kind="Internal").ap()
    tb2own = nc.dram_tensor("tb2own", [PTS, O], f16, kind="Internal").ap()
    tb2 = nc.dram_tensor("tb2", [S, O], f16, kind="Internal").ap()
    cc1_in = nc.dram_tensor("cc1_in", [128, 10], f32, kind="Internal").ap()
    cc1_out = nc.dram_tensor("cc1_out", [128, 10], f32, kind="Internal").ap()
    cc2_in = nc.dram_tensor("cc2_in", [128, 5], f32, kind="Internal").ap()
    cc2_out = nc.dram_tensor("cc2_out", [128, 5], f32, kind="Internal").ap()

    RG = [[0, 1, 2, 3], [4, 5, 6, 7]]
    RG8 = [[0, 1, 2, 3, 4, 5, 6, 7]]

    from contextlib import ExitStack
    ctx = ExitStack()
    with tile.TileContext(nc) as tc, ctx:
        persist = ctx.enter_context(tc.tile_pool(name="persist", bufs=1))
        gst_pool = ctx.enter_context(tc.tile_pool(name="gst", bufs=1))
        sc_pool = ctx.enter_context(tc.tile_pool(name="scores", bufs=2))
        wk_pool = ctx.enter_context(tc.tile_pool(name="work", bufs=2))
        ps_pool = ctx.enter_context(tc.tile_pool(name="ps", bufs=6, space="PSUM"))
        px_pool = ctx.enter_context(tc.tile_pool(name="psX", bufs=1, space="PSUM"))

        def psum(shape, tag="ps", dtp=None):
            return ps_pool.tile(shape, dtp or f32, tag=tag, name=tag)

        # ---- persistent SBUF ----
        hx_sb = persist.tile([128, HXC], f16)           # own h/x shard
        wsh_sb = persist.tile([128, WSH], f16)
        wtall_sb = persist.tile([128, WCOLS], f16)      # gathered weights
        pca_sb = persist.tile([4, S], f32)
        pcam_sb = persist.tile([4, PTS], f32)
        pct_sb = persist.tile([4, PTS], f32)
        wtg_sb = persist.tile([3, 3 * O], f32)
        bcol_sb = persist.tile([128, 3], f32)
        brow_sb = persist.tile([1, 3 * O], f32)
        hmy_sb = persist.tile([H, PTS], f32)            # own h, f32, contiguous
        idx_sb = persist.tile([128, 8 * ST], u32)
        ones16 = persist.tile([128, 1], f16)
        onesK = persist.tile([1, 128], f32)
        ident = persist.tile([128, 128], f16)
        b_bc = persist.tile([128, 3 * O], f16)
        c_cm = persist.tile([128, 3 * PTS], f16)        # c channel-major, per gate
        csum = persist.tile([128, 12], f32)             # Cs/C2 halves per gate
        m_cm = persist.tile([128, 3 * PTS], f16)        # gathered-max, channel-major
        c_pm = [persist.tile([128, 3 * O], f16, tag=f"c_pm{i}", name=f"c_pm{i}")
                for i in range(ST)]
        stats_sb = persist.tile([128, 10], f32)
        scl = persist.tile([128, 8], f32)               # istd/nbias per gate
        z_sb = persist.tile([O, PTS], f32)
        r_sb = persist.tile([O, PTS], f32)

        stats_ps = px_pool.tile([128, 96], f32)         # PE stat columns

        # ---- stage + launch the two input AllGathers first ----
        nc.sync.dma_start(out=pcam_sb, in_=ppack[0:4, 0:PTS])
        nc.sync.dma_start(out=ag0_in, in_=pcam_sb)
        nc.gpsimd.collective_compute("AllGather", mybir.AluOpType.bypass,
                                     replica_groups=RG,
                                     ins=[ag0_in], outs=[ag0_out])
        nc.sync.dma_start(out=hx_sb, in_=hx16[:, 0:HXC])
        nc.sync.dma_start(out=wsh_sb, in_=hx16[:, HXC:HXC + WSH])
        nc.sync.dma_start(out=ag2_in, in_=wsh_sb)
        nc.gpsimd.collective_compute("AllGather", mybir.AluOpType.bypass,
                                     replica_groups=RG8,
                                     ins=[ag2_in], outs=[ag2_out])

        # full pca from the gathered shards
        for rk in range(GROUP):
            nc.sync.dma_start(out=pca_sb[:, rk * PTS:(rk + 1) * PTS],
                              in_=ag0_out[rk])
        nc.sync.dma_start(out=pct_sb, in_=ppack[4:8, 0:PTS])

        nc.sync.dma_start(out=wtg_sb, in_=ppack[0:3, PTS:PTS + 3 * O])
        nc.sync.dma_start(out=bcol_sb, in_=bcol)
        nc.sync.dma_start(out=brow_sb, in_=ppack[3:4, PTS:PTS + 3 * O])

        nc.vector.memset(ones16, 1.0)
        nc.vector.memset(onesK, 1.0)
        make_identity(nc, ident[:])

        # own h -> contiguous f32 [H, PTS] (tile-deinterleave from hx_sb)
        for t in range(ST):
            nc.scalar.activation(out=hmy_sb[:, t * 128:(t + 1) * 128],
                                 in_=hx_sb[:, t * 384:t * 384 + 128],
                                 func=AF.Copy)

        # gathered weight slices -> one contiguous SBUF block
        for k in range(NCORES):
            nc.sync.dma_start(out=wtall_sb[:, k * WSH:(k + 1) * WSH],
                              in_=ag2_out[k])
        wt0_sb = wtall_sb[:, 0:3 * O]
        wt1_sb = wtall_sb[:, 3 * O:6 * O]
        wt2_sb = wtall_sb[:, 6 * O:9 * O]
        wqh_sb = wtall_sb[:, 9 * O:9 * O + O]

        # b broadcast down partitions (point-major bias): ones^T @ brow
        psb = psum([128, 3 * O])
        nc.tensor.matmul(out=psb, lhsT=onesK, rhs=brow_sb, start=True, stop=True)
        nc.scalar.activation(out=b_bc, in_=psb, func=AF.Copy)

        # ---- scores + top-4 (overlaps the AllGathers) ----
        def emit_score(st):
            srow = sc_pool.tile([128, S], f32, tag="srow", name="srow")
            for ch in range(8):
                ps = psum([128, 512])
                nc.tensor.matmul(out=ps,
                                 lhsT=pct_sb[:, st * 128:(st + 1) * 128],
                                 rhs=pca_sb[:, ch * 512:(ch + 1) * 512],
                                 start=True, stop=True)
                # negate so max8 finds the smallest distances
                nc.scalar.activation(out=srow[:, ch * 512:(ch + 1) * 512],
                                     in_=ps, func=AF.Copy, scale=-1.0)
            mx = wk_pool.tile([128, 8], f32, tag="mx8", name="mx8")
            nc.vector.max(out=mx, in_=srow)
            nc.vector.max_index(out=idx_sb[:, st * 8:st * 8 + 8],
                                in_max=mx, in_values=srow)

        # w table (z | r | q-static): OWN tiles only, from own SBUF shard;
        # the full [S, 384] table is then AllGathered (f16, 768KB in).
        def emit_own_table(st):
            pst = psum([128, 3 * O])
            nc.tensor.matmul(out=pst, lhsT=hx_sb[:, st * 384:st * 384 + 128],
                             rhs=wt0_sb, start=True, stop=False)
            nc.tensor.matmul(out=pst,
                             lhsT=hx_sb[:, st * 384 + 128:st * 384 + 256],
                             rhs=wt1_sb, start=False, stop=False)
            nc.tensor.matmul(out=pst,
                             lhsT=hx_sb[:, st * 384 + 256:st * 384 + 384],
                             rhs=wt2_sb, start=False, stop=False)
            nc.tensor.matmul(out=pst,
                             lhsT=pcam_sb[0:3, st * 128:(st + 1) * 128],
                             rhs=wtg_sb, start=False, stop=True)
            tb_sb = wk_pool.tile([128, 3 * O], f16, tag="tb_sb", name="tb_sb")
            nc.scalar.activation(out=tb_sb, in_=pst, func=AF.Copy)
            nc.sync.dma_start(out=tb1own[st * 128:(st + 1) * 128, :], in_=tb_sb)

        # scores first in the PE queue: they gate on ag0 (group AllGather)
        # which completes no later than ag2, avoiding head-of-line blocking
        for st in range(ST):
            emit_score(st)
        for st in range(ST):
            emit_own_table(st)
        nc.gpsimd.collective_compute("AllGather", mybir.AluOpType.bypass,
                                     replica_groups=RG,
                                     ins=[tb1own], outs=[tb1])

        # ---------------- c tiles ----------------
        # channel-major: c[o, s] = b[o] - v[o, s];  Cs/C2 via ScalarE accum.
        for g in range(3):
            for hh in range(2):
                psv = psum([128, 512])
                nc.tensor.matmul(out=psv,
                                 lhsT=wtg_sb[:, g * O:(g + 1) * O],
                                 rhs=pcam_sb[0:3, hh * 512:(hh + 1) * 512],
                                 start=True, stop=True)
                cs = slice(g * PTS + hh * 512, g * PTS + (hh + 1) * 512)
                nc.scalar.activation(out=c_cm[:, cs], in_=psv, func=AF.Identity,
                                     bias=bcol_sb[:, g:g + 1], scale=-1.0,
                                     accum_out=csum[:, 4 * g + hh:4 * g + hh + 1])
                scr = wk_pool.tile([128, 512], f16, tag="c2scr")
                nc.scalar.activation(out=scr, in_=psv, func=AF.Square,
                                     bias=bcol_sb[:, g:g + 1], scale=-1.0,
                                     accum_out=csum[:, 4 * g + 2 + hh:4 * g + 3 + hh])

        # point-major c tiles (for the X statistic)
        for st in range(ST):
            psv2 = psum([128, 3 * O])
            nc.tensor.matmul(out=psv2,
                             lhsT=pcam_sb[0:3, st * 128:(st + 1) * 128],
                             rhs=wtg_sb, start=True, stop=True)
            nc.scalar.activation(out=c_pm[st], in_=psv2, func=AF.Copy, scale=-1.0)
            nc.vector.tensor_add(c_pm[st], c_pm[st], b_bc)

        # ---------------- phase-1 gathers + folds (z, r) ----------------
        gtiles = [[gst_pool.tile([128, 3 * O], f16, tag=f"g{st}_{j}",
                              name=f"g{st}_{j}") for j in range(K)]
                  for st in range(ST)]
        for st in range(ST):
            g0, g1, g2, g3 = gtiles[st]
            for j in range(K):
                nc.gpsimd.indirect_dma_start(
                    out=gtiles[st][j][:], out_offset=None, in_=tb1[:, :],
                    in_offset=bass.IndirectOffsetOnAxis(
                        ap=idx_sb[:, st * 8 + j:st * 8 + j + 1], axis=0))
            zr = slice(0, 2 * O)
            t = wk_pool.tile([128, 2 * O], f16, tag="t_zr")
            nc.vector.tensor_add(t, g0[:, zr], g1[:, zr])
            nc.vector.tensor_add(t, t, g2[:, zr])
            nc.vector.tensor_add(t, t, g3[:, zr])
            m = wk_pool.tile([128, 2 * O], f16, tag="m_zr")
            nc.vector.tensor_max(m, g0[:, zr], g1[:, zr])
            nc.vector.tensor_max(m, m, g2[:, zr])
            nc.vector.tensor_max(m, m, g3[:, zr])
            t2 = wk_pool.tile([128, 2 * O], f16, tag="t2_zr")
            sq = wk_pool.tile([128, 2 * O], f16, tag="sq_zr")
            nc.scalar.activation(out=t2, in_=g0[:, zr], func=AF.Square)
            nc.scalar.activation(out=sq, in_=g1[:, zr], func=AF.Square)
            nc.vector.tensor_add(t2, t2, sq)
            nc.scalar.activation(out=sq, in_=g2[:, zr], func=AF.Square)
            nc.vector.tensor_add(t2, t2, sq)
            nc.scalar.activation(out=sq, in_=g3[:, zr], func=AF.Square)
            nc.vector.tensor_add(t2, t2, sq)
            ct = wk_pool.tile([128, 2 * O], f16, tag="ct_zr")
            nc.vector.tensor_mul(ct, c_pm[st][:, zr], t)
            for qi, srct in ((0, t), (2, t2), (4, ct)):
                for gx in range(2):
                    col = (qi + gx) * 8 + st
                    nc.tensor.matmul(out=stats_ps[:, col:col + 1],
                                     lhsT=srct[:, gx * O:(gx + 1) * O],
                                     rhs=ones16, start=True, stop=True)
            # transpose m -> channel-major
            for gx in range(2):
                ptr = psum([128, 128], dtp=f16)
                nc.tensor.transpose(out=ptr, in_=m[:, gx * O:(gx + 1) * O],
                                    identity=ident)
                nc.scalar.activation(
                    out=m_cm[:, gx * PTS + st * 128:gx * PTS + (st + 1) * 128],
                    in_=ptr, func=AF.Copy)

        # ---------------- stats AllReduce #1 (z, r) ----------------
        ccp = persist.tile([128, 10], f32)
        # cols: A B2 X Cs C2 per gate
        for gx in range(2):
            nc.vector.tensor_reduce(out=ccp[:, 5 * gx + 0:5 * gx + 1],
                                    in_=stats_ps[:, (0 + gx) * 8:(0 + gx) * 8 + 8],
                                    axis=mybir.AxisListType.X, op=ALU.add)
            nc.vector.tensor_reduce(out=ccp[:, 5 * gx + 1:5 * gx + 2],
                                    in_=stats_ps[:, (2 + gx) * 8:(2 + gx) * 8 + 8],
                                    axis=mybir.AxisListType.X, op=ALU.add)
            nc.vector.tensor_reduce(out=ccp[:, 5 * gx + 2:5 * gx + 3],
                                    in_=stats_ps[:, (4 + gx) * 8:(4 + gx) * 8 + 8],
                                    axis=mybir.AxisListType.X, op=ALU.add)
            nc.vector.tensor_add(ccp[:, 5 * gx + 3:5 * gx + 4],
                                 csum[:, 4 * gx:4 * gx + 1],
                                 csum[:, 4 * gx + 1:4 * gx + 2])
            nc.vector.tensor_add(ccp[:, 5 * gx + 4:5 * gx + 5],
                                 csum[:, 4 * gx + 2:4 * gx + 3],
                                 csum[:, 4 * gx + 3:4 * gx + 4])
        nc.sync.dma_start(out=cc1_in, in_=ccp)
        nc.gpsimd.collective_compute("AllReduce", mybir.AluOpType.add,
                                     replica_groups=RG,
                                     ins=[cc1_in], outs=[cc1_out])
        nc.sync.dma_start(out=stats_sb, in_=cc1_out)

        # ---------------- finalize gate scale/bias ----------------
        def finalize(gx, A, B2, X, Cs, C2, o_istd, o_nbias):
            w1 = wk_pool.tile([128, 1], f32, tag="fw1")
            w2 = wk_pool.tile([128, 1], f32, tag="fw2")
            w3 = wk_pool.tile([128, 1], f32, tag="fw3")
            # mu = (A + 4*Cs)/NK
            nc.vector.tensor_scalar(w1, Cs, 4.0, None, op0=ALU.mult)
            nc.vector.tensor_add(w1, w1, A)
            nc.vector.tensor_scalar(w1, w1, 1.0 / NK, None, op0=ALU.mult)
            # Ey2 = (B2 + 2X + 4*C2)/NK
            nc.vector.tensor_scalar(w2, X, 2.0, None, op0=ALU.mult)
            nc.vector.tensor_add(w2, w2, B2)
            nc.vector.tensor_scalar(w3, C2, 4.0, None, op0=ALU.mult)
            nc.vector.tensor_add(w2, w2, w3)
            nc.vector.tensor_scalar(w2, w2, 1.0 / NK, None, op0=ALU.mult)
            # var = Ey2 - mu^2 ; istd = 1/sqrt(var+eps); nbias = -mu*istd
            nc.vector.tensor_mul(w3, w1, w1)
            nc.vector.tensor_sub(w2, w2, w3)
            nc.vector.tensor_scalar_add(w2, w2, EPS)
            nc.scalar.activation(out=w2, in_=w2, func=AF.Sqrt)
            nc.vector.reciprocal(o_istd, w2)
            nc.vector.tensor_mul(o_nbias, w1, o_istd)
            nc.vector.tensor_scalar(o_nbias, o_nbias, -1.0, None, op0=ALU.mult)

        for gx in range(2):
            c0 = 5 * gx
            finalize(gx,
                     stats_sb[:, c0:c0 + 1], stats_sb[:, c0 + 1:c0 + 2],
                     stats_sb[:, c0 + 2:c0 + 3], stats_sb[:, c0 + 3:c0 + 4],
                     stats_sb[:, c0 + 4:c0 + 5],
                     scl[:, 2 * gx:2 * gx + 1], scl[:, 2 * gx + 1:2 * gx + 2])

        # ---------------- z, r gates ----------------
        for gx, dst in ((0, z_sb), (1, r_sb)):
            pre = wk_pool.tile([128, PTS], f16, tag="pre")
            nc.vector.tensor_add(pre, m_cm[:, gx * PTS:(gx + 1) * PTS],
                                 c_cm[:, gx * PTS:(gx + 1) * PTS])
            nc.scalar.activation(out=dst, in_=pre, func=AF.Sigmoid,
                                 scale=scl[:, 2 * gx:2 * gx + 1],
                                 bias=scl[:, 2 * gx + 1:2 * gx + 2])

        # ---------------- q table (dynamic part): own tiles + AllGather ----
        rh = wk_pool.tile([H, PTS], f16, tag="rh")
        nc.vector.tensor_mul(rh, r_sb, hmy_sb)
        for st in range(ST):
            ps2 = psum([128, O])
            nc.tensor.matmul(out=ps2, lhsT=rh[:, st * 128:(st + 1) * 128],
                             rhs=wqh_sb, start=True, stop=True)
            tq_sb = wk_pool.tile([128, O], f16, tag="tq_sb")
            nc.scalar.activation(out=tq_sb, in_=ps2, func=AF.Copy)
            nc.sync.dma_start(out=tb2own[st * 128:(st + 1) * 128, :], in_=tq_sb)
        nc.gpsimd.collective_compute("AllGather", mybir.AluOpType.bypass,
                                     replica_groups=RG,
                                     ins=[tb2own], outs=[tb2])

        # ---------------- phase-2 gathers + folds (q) ----------------
        qs = slice(2 * O, 3 * O)
        for st in range(ST):
            gq = [wk_pool.tile([128, O], f16, tag=f"gq{j}", name=f"gq{j}")
                  for j in range(K)]
            for j in range(K):
                nc.gpsimd.indirect_dma_start(
                    out=gq[j][:], out_offset=None, in_=tb2[:, :],
                    in_offset=bass.IndirectOffsetOnAxis(
                        ap=idx_sb[:, st * 8 + j:st * 8 + j + 1], axis=0))
                nc.vector.tensor_add(gq[j], gq[j], gtiles[st][j][:, qs])
            t = wk_pool.tile([128, O], f16, tag="t_q")
            nc.vector.tensor_add(t, gq[0], gq[1])
            nc.vector.tensor_add(t, t, gq[2])
            nc.vector.tensor_add(t, t, gq[3])
            m = wk_pool.tile([128, O], f16, tag="m_q")
            nc.vector.tensor_max(m, gq[0], gq[1])
            nc.vector.tensor_max(m, m, gq[2])
            nc.vector.tensor_max(m, m, gq[3])
            t2 = wk_pool.tile([128, O], f16, tag="t2_q")
            sq = wk_pool.tile([128, O], f16, tag="sq_q")
            nc.scalar.activation(out=t2, in_=gq[0], func=AF.Square)
            nc.scalar.activation(out=sq, in_=gq[1], func=AF.Square)
            nc.vector.tensor_add(t2, t2, sq)
            nc.scalar.activation(out=sq, in_=gq[2], func=AF.Square)
            nc.vector.tensor_add(t2, t2, sq)
            nc.scalar.activation(out=sq, in_=gq[3], func=AF.Square)
            nc.vector.tensor_add(t2, t2, sq)
            ct = wk_pool.tile([128, O], f16, tag="ct_q")
            nc.vector.tensor_mul(ct, c_pm[st][:, qs], t)
            for qi, srct in ((6, t), (7, t2), (8, ct)):
                col = qi * 8 + st
                nc.tensor.matmul(out=stats_ps[:, col:col + 1], lhsT=srct,
                                 rhs=ones16, start=True, stop=True)
            ptr = psum([128, 128], dtp=f16)
            nc.tensor.transpose(out=ptr, in_=m, identity=ident)
            nc.scalar.activation(
                out=m_cm[:, 2 * PTS + st * 128:2 * PTS + (st + 1) * 128],
                in_=ptr, func=AF.Copy)

        # ---------------- stats AllReduce #2 (q) ----------------
        ccq = persist.tile([128, 5], f32)
        nc.vector.tensor_reduce(out=ccq[:, 0:1], in_=stats_ps[:, 48:56],
                                axis=mybir.AxisListType.X, op=ALU.add)
        nc.vector.tensor_reduce(out=ccq[:, 1:2], in_=stats_ps[:, 56:64],
                                axis=mybir.AxisListType.X, op=ALU.add)
        nc.vector.tensor_reduce(out=ccq[:, 2:3], in_=stats_ps[:, 64:72],
                                axis=mybir.AxisListType.X, op=ALU.add)
        nc.vector.tensor_add(ccq[:, 3:4], csum[:, 8:9], csum[:, 9:10])
        nc.vector.tensor_add(ccq[:, 4:5], csum[:, 10:11], csum[:, 11:12])
        nc.sync.dma_start(out=cc2_in, in_=ccq)
        nc.gpsimd.collective_compute("AllReduce", mybir.AluOpType.add,
                                     replica_groups=RG,
                                     ins=[cc2_in], outs=[cc2_out])
        stats2 = persist.tile([128, 5], f32)
        nc.sync.dma_start(out=stats2, in_=cc2_out)
        finalize(2, stats2[:, 0:1], stats2[:, 1:2], stats2[:, 2:3],
                 stats2[:, 3:4], stats2[:, 4:5],
                 scl[:, 4:5], scl[:, 5:6])

        # ---------------- q gate + output ----------------
        qpre = wk_pool.tile([128, PTS], f16, tag="qpre")
        nc.vector.tensor_add(qpre, m_cm[:, 2 * PTS:3 * PTS],
                             c_cm[:, 2 * PTS:3 * PTS])
        q_sb = persist.tile([O, PTS], f32)
        nc.scalar.activation(out=q_sb, in_=qpre, func=AF.Tanh,
                             scale=scl[:, 4:5], bias=scl[:, 5:6])
        # delta = z*(q-h), 12-bit per-channel quantized (h re-added on host).
        # q = RTN(delta*2040/amax) in [-2040,2040]; shifted positive
        # qs = q+2048, split qs = hs*256 + r with plain mod (operands >= 0),
        # low byte r-128 -> int8, two 4-bit highs packed per byte.
        dfin = persist.tile([O, PTS], f32)
        nc.vector.tensor_sub(dfin, q_sb, hmy_sb)
        nc.vector.tensor_mul(dfin, dfin, z_sb)
        dabs = wk_pool.tile([O, PTS], f32, tag="dabs")
        nc.scalar.activation(out=dabs, in_=dfin, func=AF.Abs)
        amax = persist.tile([O, 1], f32)
        nc.vector.tensor_reduce(out=amax, in_=dabs,
                                axis=mybir.AxisListType.X, op=ALU.max)
        nc.vector.tensor_scalar_add(amax, amax, 1e-12)
        qscl = persist.tile([O, 1], f32)
        nc.vector.reciprocal(qscl, amax)
        nc.vector.tensor_scalar(qscl, qscl, 2040.0, None, op0=ALU.mult)
        q16 = wk_pool.tile([O, PTS], dt.int16, tag="q16")
        nc.scalar.activation(out=q16, in_=dfin, func=AF.Copy, scale=qscl)
        qs = wk_pool.tile([O, PTS], f32, tag="qs")
        nc.scalar.activation(out=qs, in_=q16, func=AF.Copy, bias=2048.0)
        # hs = floor(qs/256) via RTN((qs-127.5)/256): int convert rounds to
        # nearest and the argument is strictly inside half-integer bounds
        hs16 = wk_pool.tile([O, PTS], dt.int16, tag="hs16")
        nc.scalar.activation(out=hs16, in_=q16, func=AF.Copy,
                             scale=1.0 / 256.0, bias=(2048.0 - 127.5) / 256.0)
        hsf = wk_pool.tile([O, PTS], f32, tag="hsf")
        nc.scalar.activation(out=hsf, in_=hs16, func=AF.Copy)
        r = wk_pool.tile([O, PTS], f32, tag="rlow")
        nc.vector.tensor_scalar(r, hsf, -256.0, None, op0=ALU.mult)
        nc.vector.tensor_add(r, r, qs)
        pk8 = persist.tile([O, PTS + PTS // 2], dt.int8)
        nc.vector.tensor_scalar_add(r, r, -128.0)
        nc.scalar.activation(out=pk8[:, 0:PTS], in_=r, func=AF.Copy)
        pe = wk_pool.tile([O, PTS // 2], f32, tag="pe")
        nc.vector.tensor_scalar(pe, hsf[:, 0:PTS // 2], 16.0, None,
                                op0=ALU.mult)
        nc.vector.tensor_add(pe, pe, hsf[:, PTS // 2:PTS])
        nc.vector.tensor_scalar_add(pe, pe, -128.0)
        nc.scalar.activation(out=pk8[:, PTS:PTS + PTS // 2], in_=pe,
                             func=AF.Copy)
        nc.sync.dma_start(out=out_io, in_=pk8)
        dscl = persist.tile([O, 1], f32)
        nc.vector.tensor_scalar(dscl, amax, 1.0 / 2040.0, None, op0=ALU.mult)
        nc.sync.dma_start(out=osc_io, in_=dscl)

    nc.compile()
    return nc


def _prep_inputs(h, x, pc, Wz, bz, Wr, br, Wq, bq):
    """Host-side slicing/stacking -> per-core in_maps (minimal bytes)."""
    f32, f16 = np.float32, np.float16
    h, x, pc = np.asarray(h), np.asarray(x), np.asarray(pc)
    Wz, Wr, Wq = np.asarray(Wz), np.asarray(Wr), np.asarray(Wq)
    bz, br, bq = np.asarray(bz), np.asarray(br), np.asarray(bq)
    # stacked transposed weights [387, 384]; q's h-block removed (added in ph2)
    Wq_m = np.array(Wq, copy=True)
    Wq_m[:, 3:3 + H] = 0.0
    WT = np.concatenate([Wz.T, Wr.T, Wq_m.T], axis=1).astype(f32)  # [387, 384]
    # f16 weight block [128, 1280] = [wt0 | wt1 | wt2 | wqh]
    W16 = np.concatenate([WT[3:131], WT[131:259], WT[259:387],
                          Wq[:, 3:3 + H].T], axis=1).astype(f16)
    bcol = np.stack([bz, br, bq], axis=1).astype(f32)              # [128, 3]
    # wtg rows + stacked-bias row, appended to ppack cols PTS: of rows 0:4
    wb4 = np.concatenate([WT[0:3],
                          np.concatenate([bz, br, bq])[None, :]],
                         axis=0).astype(f32)                       # [4, 384]

    h16 = h.astype(f16)
    x16 = x.astype(f16)

    sq = (pc * pc).sum(axis=1, keepdims=True)                      # [B, 1, S]
    pca_full = np.concatenate([pc, sq], axis=1).astype(f32)        # [B, 4, S]

    in_maps = []
    for core in range(NCORES):
        b = core // GROUP
        r0 = (core % GROUP) * PTS
        # per-tile interleave [h|x0|x1]: [128, 8, 3, 128] -> [128, 3072],
        # then the core's 1/8 weight-table slice appended
        hx = np.empty((128, HXC + WSH), f16)
        hx[:, :HXC] = np.stack(
            [h16[b][:, r0:r0 + PTS].reshape(H, ST, 128),
             x16[b][:128, r0:r0 + PTS].reshape(128, ST, 128),
             x16[b][128:, r0:r0 + PTS].reshape(128, ST, 128)],
            axis=2).reshape(128, HXC)
        hx[:, HXC:] = W16[:, core * WSH:(core + 1) * WSH]
        ppack = np.zeros((8, PTS + 3 * O), f32)
        ppack[0:4, 0:PTS] = pca_full[b][:, r0:r0 + PTS]
        ppack[4:7, 0:PTS] = -2.0 * pc[b][:, r0:r0 + PTS]
        ppack[7, 0:PTS] = 1.0
        ppack[0:4, PTS:] = wb4
        in_maps.append({
            "hx16": hx,
            "ppack": ppack,
            "bcol": bcol,
        })
    return in_maps


def _enable_jax_compile_cache():
    """Persistent XLA compilation cache: run_bass_kernel_spmd re-lowers and
    re-compiles the NEFF-wrapped executable on every call (fresh jit closure
    per call); with the cache enabled, repeat calls hit the on-disk entry and
    skip ~0.2s of backend compile + BIR verify per call."""
    import os
    import tempfile
    import jax
    cache_dir = os.path.join(tempfile.gettempdir(), "jax_comp_cache_kernel")
    try:
        jax.config.update("jax_compilation_cache_dir", cache_dir)
        jax.config.update("jax_persistent_cache_min_compile_time_secs", 0.0)
        jax.config.update("jax_persistent_cache_min_entry_size_bytes", 0)
    except Exception:
        pass  # older jax without these flags: run uncached


def kernel(h, x, pc, Wz, bz, Wr, br, Wq, bq):
    from concourse.bass_utils import run_bass_kernel_spmd
    if "nc" not in _CACHE:
        _enable_jax_compile_cache()
        _CACHE["nc"] = _build_program()
    nc = _CACHE["nc"]
    h = np.asarray(h, dtype=np.float32)
    in_maps = _prep_inputs(h, x, pc, Wz, bz, Wr, br, Wq, bq)
    res = run_bass_kernel_spmd(nc, in_maps, core_ids=list(range(NCORES)))
    _CACHE["last_results"] = res
    out = np.zeros((B, H, S), np.float32)
    for core in range(NCORES):
        b = core // GROUP
        r0 = (core % GROUP) * PTS
        pk = res.results[core]["out"].astype(np.int32)
        r = pk[:, 0:PTS] + 128                       # [0,255] low byte
        pu = pk[:, PTS:PTS + PTS // 2] + 128         # [0,255] packed highs
        hs = np.concatenate([pu >> 4, pu & 0xF], axis=1)  # [0,15] highs
        q = (hs * 256 + r - 2048).astype(np.float32)
        delta = q * res.results[core]["osc"]
        out[b][:, r0:r0 + PTS] = h[b][:, r0:r0 + PTS] + delta
    return out


# revision 46
# speedup vs baseline: 1.1730x; 1.1730x over previous
"""Trainium2 Bass kernel for point-cloud GRU (kNN set-conv gates, InstanceNorm).

Wall time through the axon tunnel is dominated by host<->device transfer
and per-call dispatch, not device compute, so the design minimizes moved
bytes and per-call work:
  - 4 cores per batch (B=2), each owning a 1024-point shard of S=4096.
  - Each core receives ONLY its own f16 h/x shard (tile-interleaved),
    a 1/8 slice of the f16 weight tables, and tiny f32 packs; ~0.9MB/core
    vs ~7.3MB/core for replicated-f32 inputs.
  - The per-point projection tables (set-conv linearization, below) are
    computed per-shard from the core's own data, then AllGathered (f16)
    within the batch group -- cheaper and shorter-chained than gathering
    raw h/x and computing full tables on every core.
  - Output is the f16 GRU delta z*(q-h) only; full-precision h is added
    back on the host.
  - A persistent XLA compilation cache is enabled: run_bass_kernel_spmd
    re-lowers+re-compiles its jit closure every call otherwise (~0.2s).

Device algorithm:
  - kNN (k=4): PE computes score[i,j] = |x_j|^2 - 2 x_i.x_j for own rows,
    DVE max8+max_index on negated scores -> 4 smallest (self included).
  - Set-conv is linearized: y[s,k,o] = w[idx[s,k], o] + c[o, s] where
    w[n,o] = W_feat.f[n] + W_xyz.xyz[n] (per-point projection table) and
    c[o,s] = b[o] - W_xyz.xyz[s].  Table rows gathered by neighbor index
    (SWDGE indirect DMA) from the AllGathered [S, 384] f16 table.
  - InstanceNorm stats over (S,k) per (b,o) from algebraic identities:
      sum y   = A + k*Cs,   A  = sum_s t[s],  t = sum_k w[idx[s,k]]
      sum y^2 = B2 + 2*X + k*C2,  B2 = sum_s sum_k w^2,  X = sum_s c.t
    A/B2/X via PE ones-matmuls; Cs/C2 via ScalarE accum; partials
    AllReduced across the 4-core batch group (tiny).
  - max_k commutes with the (monotonic) normalization: out uses m = max_k w.
  - Phase 2 (q gate) needs r at neighbor points: per-shard q-table tiles
    Wq_h.(r*h) are computed locally then AllGathered (f16, [S, 128]).
"""

import numpy as np

B, S, H, D = 2, 4096, 128, 256
O = 128
K = 4
NCORES = 8
GROUP = 4              # cores per batch
PTS = S // GROUP       # points per core
NT = S // 128          # 32 table M-tiles
ST = PTS // 128        # 8 own s-tiles
EPS = 1e-5
NK = float(S * K)
WCOLS = 3 * (3 * O) + O        # wt0|wt1|wt2|wqh stacked -> 1280 f16 cols
WSH = WCOLS // 8               # 1280 / 8 = 160
HXC = 3 * PTS          # 3072 f16 cols: per-tile [h|x0|x1] interleave

_CACHE = {}


def _build_program():
    from concourse import bass, bacc, mybir, tile
    from concourse.masks import make_identity

    dt = mybir.dt
    f32, f16, u32 = dt.float32, dt.float16, dt.uint32
    AF = mybir.ActivationFunctionType
    ALU = mybir.AluOpType

    nc = bacc.Bacc("TRN2", target_bir_lowering=False, debug=False,
                   enable_asserts=False, num_devices=NCORES)

    # ---------------- I/O ----------------
    # hx16 cols: 0:HXC tile-interleaved [h|x0|x1] shard, HXC:HXC+WSH weight slice
    hx16 = nc.dram_tensor("hx16", [128, HXC + WSH], f16,
                          kind="ExternalInput").ap()
    # ppack rows: 0:4 pca_my ([pc;sq] own shard), 4:8 pct ([-2pc;ones]);
    # cols PTS:PTS+3*O of rows 0:4 hold wtg (3 rows) + brow (row 3)
    ppack = nc.dram_tensor("ppack", [8, PTS + 3 * O], f32,
                           kind="ExternalInput").ap()
    bcol = nc.dram_tensor("bcol", [128, 3], f32, kind="ExternalInput").ap()
    out_io = nc.dram_tensor("out", [O, PTS], f16, kind="ExternalOutput").ap()

    # ---------------- internal DRAM ----------------
    ag0_in = nc.dram_tensor("ag0_in", [4, PTS], f32, kind="Internal").ap()
    ag0_out = nc.dram_tensor("ag0_out", [GROUP, 4, PTS], f32,
                             kind="Internal").ap()
    ag2_in = nc.dram_tensor("ag2_in", [128, WSH], f16, kind="Internal").ap()
    ag2_out = nc.dram_tensor("ag2_out", [NCORES, 128, WSH], f16,
                             kind="Internal").ap()
    tb1own = nc.dram_tensor("tb1own", [PTS, 3 * O], f16, kind="Internal").ap()
    tb1 = nc.dram_tensor("tb1", [S, 3 * O], f16, kind="Internal").ap()
    tb2own = nc.dram_tensor("tb2own", [PTS, O], f16, kind="Internal").ap()
    tb2 = nc.dram_tensor("tb2", [S, O], f16, kind="Internal").ap()
    cc1_in = nc.dram_tensor("cc1_in", [128, 10], f32, kind="Internal").ap()
    cc1_out = nc.dram_tensor("cc1_out", [128, 10], f32, kind="Internal").ap()
    cc2_in = nc.dram_tensor("cc2_in", [128, 5], f32, kind="Internal").ap()
    cc2_out = nc.dram_tensor("cc2_out", [128, 5], f32, kind="Internal").ap()

    RG = [[0, 1, 2, 3], [4, 5, 6, 7]]
    RG8 = [[0, 1, 2, 3, 4, 5, 6, 7]]

    from contextlib import ExitStack
    ctx = ExitStack()
    with tile.TileContext(nc) as tc, ctx:
        persist = ctx.enter_context(tc.tile_pool(name="persist", bufs=1))
        gst_pool = ctx.enter_context(tc.tile_pool(name="gst", bufs=1))
        sc_pool = ctx.enter_context(tc.tile_pool(name="scores", bufs=2))
        wk_pool = ctx.enter_context(tc.tile_pool(name="work", bufs=2))
        ps_pool = ctx.enter_context(tc.tile_pool(name="ps", bufs=6, space="PSUM"))
        px_pool = ctx.enter_context(tc.tile_pool(name="psX", bufs=1, space="PSUM"))

        def psum(shape, tag="ps", dtp=None):
            return ps_pool.tile(shape, dtp or f32, tag=tag, name=tag)

        # ---- persistent SBUF ----
        hx_sb = persist.tile([128, HXC], f16)           # own h/x shard
        wsh_sb = persist.tile([128, WSH], f16)
        wtall_sb = persist.tile([128, WCOLS], f16)      # gathered weights
        pca_sb = persist.tile([4, S], f32)
        pcam_sb = persist.tile([4, PTS], f32)
        pct_sb = persist.tile([4, PTS], f32)
        wtg_sb = persist.tile([3, 3 * O], f32)
        bcol_sb = persist.tile([128, 3], f32)
        brow_sb = persist.tile([1, 3 * O], f32)
        hmy_sb = persist.tile([H, PTS], f32)            # own h, f32, contiguous
        idx_sb = persist.tile([128, 8 * ST], u32)
        ones16 = persist.tile([128, 1], f16)
        onesK = persist.tile([1, 128], f32)
        ident = persist.tile([128, 128], f16)
        b_bc = persist.tile([128, 3 * O], f16)
        c_cm = persist.tile([128, 3 * PTS], f16)        # c channel-major, per gate
        csum = persist.tile([128, 12], f32)             # Cs/C2 halves per gate
        m_cm = persist.tile([128, 3 * PTS], f16)        # gathered-max, channel-major
        c_pm = [persist.tile([128, 3 * O], f16, tag=f"c_pm{i}", name=f"c_pm{i}")
                for i in range(ST)]
        stats_sb = persist.tile([128, 10], f32)
        scl = persist.tile([128, 8], f32)               # istd/nbias per gate
        z_sb = persist.tile([O, PTS], f32)
        r_sb = persist.tile([O, PTS], f32)

        stats_ps = px_pool.tile([128, 96], f32)         # PE stat columns

        # ---- stage + launch the two input AllGathers first ----
        nc.sync.dma_start(out=pcam_sb, in_=ppack[0:4, 0:PTS])
        nc.sync.dma_start(out=ag0_in, in_=pcam_sb)
        nc.gpsimd.collective_compute("AllGather", mybir.AluOpType.bypass,
                                     replica_groups=RG,
                                     ins=[ag0_in], outs=[ag0_out])
        nc.sync.dma_start(out=hx_sb, in_=hx16[:, 0:HXC])
        nc.sync.dma_start(out=wsh_sb, in_=hx16[:, HXC:HXC + WSH])
        nc.sync.dma_start(out=ag2_in, in_=wsh_sb)
        nc.gpsimd.collective_compute("AllGather", mybir.AluOpType.bypass,
                                     replica_groups=RG8,
                                     ins=[ag2_in], outs=[ag2_out])

        # full pca from the gathered shards
        for rk in range(GROUP):
            nc.sync.dma_start(out=pca_sb[:, rk * PTS:(rk + 1) * PTS],
                              in_=ag0_out[rk])
        nc.sync.dma_start(out=pct_sb, in_=ppack[4:8, 0:PTS])

        nc.sync.dma_start(out=wtg_sb, in_=ppack[0:3, PTS:PTS + 3 * O])
        nc.sync.dma_start(out=bcol_sb, in_=bcol)
        nc.sync.dma_start(out=brow_sb, in_=ppack[3:4, PTS:PTS + 3 * O])

        nc.vector.memset(ones16, 1.0)
        nc.vector.memset(onesK, 1.0)
        make_identity(nc, ident[:])

        # own h -> contiguous f32 [H, PTS] (tile-deinterleave from hx_sb)
        for t in range(ST):
            nc.scalar.activation(out=hmy_sb[:, t * 128:(t + 1) * 128],
                                 in_=hx_sb[:, t * 384:t * 384 + 128],
                                 func=AF.Copy)

        # gathered weight slices -> one contiguous SBUF block
        for k in range(NCORES):
            nc.sync.dma_start(out=wtall_sb[:, k * WSH:(k + 1) * WSH],
                              in_=ag2_out[k])
        wt0_sb = wtall_sb[:, 0:3 * O]
        wt1_sb = wtall_sb[:, 3 * O:6 * O]
        wt2_sb = wtall_sb[:, 6 * O:9 * O]
        wqh_sb = wtall_sb[:, 9 * O:9 * O + O]

        # b broadcast down partitions (point-major bias): ones^T @ brow
        psb = psum([128, 3 * O])
        nc.tensor.matmul(out=psb, lhsT=onesK, rhs=brow_sb, start=True, stop=True)
        nc.scalar.activation(out=b_bc, in_=psb, func=AF.Copy)

        # ---- scores + top-4 (overlaps the AllGathers) ----
        def emit_score(st):
            srow = sc_pool.tile([128, S], f32, tag="srow", name="srow")
            for ch in range(8):
                ps = psum([128, 512])
                nc.tensor.matmul(out=ps,
                                 lhsT=pct_sb[:, st * 128:(st + 1) * 128],
                                 rhs=pca_sb[:, ch * 512:(ch + 1) * 512],
                                 start=True, stop=True)
                # negate so max8 finds the smallest distances
                nc.scalar.activation(out=srow[:, ch * 512:(ch + 1) * 512],
                                     in_=ps, func=AF.Copy, scale=-1.0)
            mx = wk_pool.tile([128, 8], f32, tag="mx8", name="mx8")
            nc.vector.max(out=mx, in_=srow)
            nc.vector.max_index(out=idx_sb[:, st * 8:st * 8 + 8],
                                in_max=mx, in_values=srow)

        # w table (z | r | q-static): OWN tiles only, from own SBUF shard;
        # the full [S, 384] table is then AllGathered (f16, 768KB in).
        def emit_own_table(st):
            pst = psum([128, 3 * O])
            nc.tensor.matmul(out=pst, lhsT=hx_sb[:, st * 384:st * 384 + 128],
                             rhs=wt0_sb, start=True, stop=False)
            nc.tensor.matmul(out=pst,
                             lhsT=hx_sb[:, st * 384 + 128:st * 384 + 256],
                             rhs=wt1_sb, start=False, stop=False)
            nc.tensor.matmul(out=pst,
                             lhsT=hx_sb[:, st * 384 + 256:st * 384 + 384],
                             rhs=wt2_sb, start=False, stop=False)
            nc.tensor.matmul(out=pst,
                             lhsT=pcam_sb[0:3, st * 128:(st + 1) * 128],
                             rhs=wtg_sb, start=False, stop=True)
            tb_sb = wk_pool.tile([128, 3 * O], f16, tag="tb_sb", name="tb_sb")
            nc.scalar.activation(out=tb_sb, in_=pst, func=AF.Copy)
            nc.sync.dma_start(out=tb1own[st * 128:(st + 1) * 128, :], in_=tb_sb)

        # scores first in the PE queue: they gate on ag0 (group AllGather)
        # which completes no later than ag2, avoiding head-of-line blocking
        for st in range(ST):
            emit_score(st)
        for st in range(ST):
            emit_own_table(st)
        nc.gpsimd.collective_compute("AllGather", mybir.AluOpType.bypass,
                                     replica_groups=RG,
                                     ins=[tb1own], outs=[tb1])

        # ---------------- c tiles ----------------
        # channel-major: c[o, s] = b[o] - v[o, s];  Cs/C2 via ScalarE accum.
        for g in range(3):
            for hh in range(2):
                psv = psum([128, 512])
                nc.tensor.matmul(out=psv,
                                 lhsT=wtg_sb[:, g * O:(g + 1) * O],
                                 rhs=pcam_sb[0:3, hh * 512:(hh + 1) * 512],
                                 start=True, stop=True)
                cs = slice(g * PTS + hh * 512, g * PTS + (hh + 1) * 512)
                nc.scalar.activation(out=c_cm[:, cs], in_=psv, func=AF.Identity,
                                     bias=bcol_sb[:, g:g + 1], scale=-1.0,
                                     accum_out=csum[:, 4 * g + hh:4 * g + hh + 1])
                scr = wk_pool.tile([128, 512], f16, tag="c2scr")
                nc.scalar.activation(out=scr, in_=psv, func=AF.Square,
                                     bias=bcol_sb[:, g:g + 1], scale=-1.0,
                                     accum_out=csum[:, 4 * g + 2 + hh:4 * g + 3 + hh])

        # point-major c tiles (for the X statistic)
        for st in range(ST):
            psv2 = psum([128, 3 * O])
            nc.tensor.matmul(out=psv2,
                             lhsT=pcam_sb[0:3, st * 128:(st + 1) * 128],
                             rhs=wtg_sb, start=True, stop=True)
            nc.scalar.activation(out=c_pm[st], in_=psv2, func=AF.Copy, scale=-1.0)
            nc.vector.tensor_add(c_pm[st], c_pm[st], b_bc)

        # ---------------- phase-1 gathers + folds (z, r) ----------------
        gtiles = [[gst_pool.tile([128, 3 * O], f16, tag=f"g{st}_{j}",
                              name=f"g{st}_{j}") for j in range(K)]
                  for st in range(ST)]
        for st in range(ST):
            g0, g1, g2, g3 = gtiles[st]
            for j in range(K):
                nc.gpsimd.indirect_dma_start(
                    out=gtiles[st][j][:], out_offset=None, in_=tb1[:, :],
                    in_offset=bass.IndirectOffsetOnAxis(
                        ap=idx_sb[:, st * 8 + j:st * 8 + j + 1], axis=0))
            zr = slice(0, 2 * O)
            t = wk_pool.tile([128, 2 * O], f16, tag="t_zr")
            nc.vector.tensor_add(t, g0[:, zr], g1[:, zr])
            nc.vector.tensor_add(t, t, g2[:, zr])
            nc.vector.tensor_add(t, t, g3[:, zr])
            m = wk_pool.tile([128, 2 * O], f16, tag="m_zr")
            nc.vector.tensor_max(m, g0[:, zr], g1[:, zr])
            nc.vector.tensor_max(m, m, g2[:, zr])
            nc.vector.tensor_max(m, m, g3[:, zr])
            t2 = wk_pool.tile([128, 2 * O], f16, tag="t2_zr")
            sq = wk_pool.tile([128, 2 * O], f16, tag="sq_zr")
            nc.scalar.activation(out=t2, in_=g0[:, zr], func=AF.Square)
            nc.scalar.activation(out=sq, in_=g1[:, zr], func=AF.Square)
            nc.vector.tensor_add(t2, t2, sq)
            nc.scalar.activation(out=sq, in_=g2[:, zr], func=AF.Square)
            nc.vector.tensor_add(t2, t2, sq)
            nc.scalar.activation(out=sq, in_=g3[:, zr], func=AF.Square)
            nc.vector.tensor_add(t2, t2, sq)
            ct = wk_pool.tile([128, 2 * O], f16, tag="ct_zr")
            nc.vector.tensor_mul(ct, c_pm[st][:, zr], t)
            for qi, srct in ((0, t), (2, t2), (4, ct)):
                for gx in range(2):
                    col = (qi + gx) * 8 + st
                    nc.tensor.matmul(out=stats_ps[:, col:col + 1],
                                     lhsT=srct[:, gx * O:(gx + 1) * O],
                                     rhs=ones16, start=True, stop=True)
            # transpose m -> channel-major
            for gx in range(2):
                ptr = psum([128, 128], dtp=f16)
                nc.tensor.transpose(out=ptr, in_=m[:, gx * O:(gx + 1) * O],
                                    identity=ident)
                nc.scalar.activation(
                    out=m_cm[:, gx * PTS + st * 128:gx * PTS + (st + 1) * 128],
                    in_=ptr, func=AF.Copy)

        # ---------------- stats AllReduce #1 (z, r) ----------------
        ccp = persist.tile([128, 10], f32)
        # cols: A B2 X Cs C2 per gate
        for gx in range(2):
            nc.vector.tensor_reduce(out=ccp[:, 5 * gx + 0:5 * gx + 1],
                                    in_=stats_ps[:, (0 + gx) * 8:(0 + gx) * 8 + 8],
                                    axis=mybir.AxisListType.X, op=ALU.add)
            nc.vector.tensor_reduce(out=ccp[:, 5 * gx + 1:5 * gx + 2],
                                    in_=stats_ps[:, (2 + gx) * 8:(2 + gx) * 8 + 8],
                                    axis=mybir.AxisListType.X, op=ALU.add)
            nc.vector.tensor_reduce(out=ccp[:, 5 * gx + 2:5 * gx + 3],
                                    in_=stats_ps[:, (4 + gx) * 8:(4 + gx) * 8 + 8],
                                    axis=mybir.AxisListType.X, op=ALU.add)
            nc.vector.tensor_add(ccp[:, 5 * gx + 3:5 * gx + 4],
                                 csum[:, 4 * gx:4 * gx + 1],
                                 csum[:, 4 * gx + 1:4 * gx + 2])
            nc.vector.tensor_add(ccp[:, 5 * gx + 4:5 * gx + 5],
                                 csum[:, 4 * gx + 2:4 * gx + 3],
                                 csum[:, 4 * gx + 3:4 * gx + 4])
        nc.sync.dma_start(out=cc1_in, in_=ccp)
        nc.gpsimd.collective_compute("AllReduce", mybir.AluOpType.add,
                                     replica_groups=RG,
                                     ins=[cc1_in], outs=[cc1_out])
        nc.sync.dma_start(out=stats_sb, in_=cc1_out)

        # ---------------- finalize gate scale/bias ----------------
        def finalize(gx, A, B2, X, Cs, C2, o_istd, o_nbias):
            w1 = wk_pool.tile([128, 1], f32, tag="fw1")
            w2 = wk_pool.tile([128, 1], f32, tag="fw2")
            w3 = wk_pool.tile([128, 1], f32, tag="fw3")
            # mu = (A + 4*Cs)/NK
            nc.vector.tensor_scalar(w1, Cs, 4.0, None, op0=ALU.mult)
            nc.vector.tensor_add(w1, w1, A)
            nc.vector.tensor_scalar(w1, w1, 1.0 / NK, None, op0=ALU.mult)
            # Ey2 = (B2 + 2X + 4*C2)/NK
            nc.vector.tensor_scalar(w2, X, 2.0, None, op0=ALU.mult)
            nc.vector.tensor_add(w2, w2, B2)
            nc.vector.tensor_scalar(w3, C2, 4.0, None, op0=ALU.mult)
            nc.vector.tensor_add(w2, w2, w3)
            nc.vector.tensor_scalar(w2, w2, 1.0 / NK, None, op0=ALU.mult)
            # var = Ey2 - mu^2 ; istd = 1/sqrt(var+eps); nbias = -mu*istd
            nc.vector.tensor_mul(w3, w1, w1)
            nc.vector.tensor_sub(w2, w2, w3)
            nc.vector.tensor_scalar_add(w2, w2, EPS)
            nc.scalar.activation(out=w2, in_=w2, func=AF.Sqrt)
            nc.vector.reciprocal(o_istd, w2)
            nc.vector.tensor_mul(o_nbias, w1, o_istd)
            nc.vector.tensor_scalar(o_nbias, o_nbias, -1.0, None, op0=ALU.mult)

        for gx in range(2):
            c0 = 5 * gx
            finalize(gx,
                     stats_sb[:, c0:c0 + 1], stats_sb[:, c0 + 1:c0 + 2],
                     stats_sb[:, c0 + 2:c0 + 3], stats_sb[:, c0 + 3:c0 + 4],
                     stats_sb[:, c0 + 4:c0 + 5],
                     scl[:, 2 * gx:2 * gx + 1], scl[:, 2 * gx + 1:2 * gx + 2])

        # ---------------- z, r gates ----------------
        for gx, dst in ((0, z_sb), (1, r_sb)):
            pre = wk_pool.tile([128, PTS], f16, tag="pre")
            nc.vector.tensor_add(pre, m_cm[:, gx * PTS:(gx + 1) * PTS],
                                 c_cm[:, gx * PTS:(gx + 1) * PTS])
            nc.scalar.activation(out=dst, in_=pre, func=AF.Sigmoid,
                                 scale=scl[:, 2 * gx:2 * gx + 1],
                                 bias=scl[:, 2 * gx + 1:2 * gx + 2])

        # ---------------- q table (dynamic part): own tiles + AllGather ----
        rh = wk_pool.tile([H, PTS], f16, tag="rh")
        nc.vector.tensor_mul(rh, r_sb, hmy_sb)
        for st in range(ST):
            ps2 = psum([128, O])
            nc.tensor.matmul(out=ps2, lhsT=rh[:, st * 128:(st + 1) * 128],
                             rhs=wqh_sb, start=True, stop=True)
            tq_sb = wk_pool.tile([128, O], f16, tag="tq_sb")
            nc.scalar.activation(out=tq_sb, in_=ps2, func=AF.Copy)
            nc.sync.dma_start(out=tb2own[st * 128:(st + 1) * 128, :], in_=tq_sb)
        nc.gpsimd.collective_compute("AllGather", mybir.AluOpType.bypass,
                                     replica_groups=RG,
                                     ins=[tb2own], outs=[tb2])

        # ---------------- phase-2 gathers + folds (q) ----------------
        qs = slice(2 * O, 3 * O)
        for st in range(ST):
            gq = [wk_pool.tile([128, O], f16, tag=f"gq{j}", name=f"gq{j}")
                  for j in range(K)]
            for j in range(K):
                nc.gpsimd.indirect_dma_start(
                    out=gq[j][:], out_offset=None, in_=tb2[:, :],
                    in_offset=bass.IndirectOffsetOnAxis(
                        ap=idx_sb[:, st * 8 + j:st * 8 + j + 1], axis=0))
                nc.vector.tensor_add(gq[j], gq[j], gtiles[st][j][:, qs])
            t = wk_pool.tile([128, O], f16, tag="t_q")
            nc.vector.tensor_add(t, gq[0], gq[1])
            nc.vector.tensor_add(t, t, gq[2])
            nc.vector.tensor_add(t, t, gq[3])
            m = wk_pool.tile([128, O], f16, tag="m_q")
            nc.vector.tensor_max(m, gq[0], gq[1])
            nc.vector.tensor_max(m, m, gq[2])
            nc.vector.tensor_max(m, m, gq[3])
            t2 = wk_pool.tile([128, O], f16, tag="t2_q")
            sq = wk_pool.tile([128, O], f16, tag="sq_q")
            nc.scalar.activation(out=t2, in_=gq[0], func=AF.Square)
            nc.scalar.activation(out=sq, in_=gq[1], func=AF.Square)
            nc.vector.tensor_add(t2, t2, sq)
            nc.scalar.activation(out=sq, in_=gq[2], func=AF.Square)
            nc.vector.tensor_add(t2, t2, sq)
            nc.scalar.activation(out=sq, in_=gq[3], func=AF.Square)
            nc.vector.tensor_add(t2, t2, sq)
            ct = wk_pool.tile([128, O], f16, tag="ct_q")
            nc.vector.tensor_mul(ct, c_pm[st][:, qs], t)
            for qi, srct in ((6, t), (7, t2), (8, ct)):
                col = qi * 8 + st
                nc.tensor.matmul(out=stats_ps[:, col:col + 1], lhsT=srct,
                                 rhs=ones16, start=True, stop=True)
            ptr = psum([128, 128], dtp=f16)
            nc.tensor.transpose(out=ptr, in_=m, identity=ident)
            nc.scalar.activation(
                out=m_cm[:, 2 * PTS + st * 128:2 * PTS + (st + 1) * 128],
                in_=ptr, func=AF.Copy)

        # ---------------- stats AllReduce #2 (q) ----------------
        ccq = persist.tile([128, 5], f32)
        nc.vector.tensor_reduce(out=ccq[:, 0:1], in_=stats_ps[:, 48:56],
                                axis=mybir.AxisListType.X, op=ALU.add)
        nc.vector.tensor_reduce(out=ccq[:, 1:2], in_=stats_ps[:, 56:64],
                                axis=mybir.AxisListType.X, op=ALU.add)
        nc.vector.tensor_reduce(out=ccq[:, 2:3], in_=stats_ps[:, 64:72],
                                axis=mybir.AxisListType.X, op=ALU.add)
        nc.vector.tensor_add(ccq[:, 3:4], csum[:, 8:9], csum[:, 9:10])
        nc.vector.tensor_add(ccq[:, 4:5], csum[:, 10:11], csum[:, 11:12])
        nc.sync.dma_start(out=cc2_in, in_=ccq)
        nc.gpsimd.collective_compute("AllReduce", mybir.AluOpType.add,
                                     replica_groups=RG,
                                     ins=[cc2_in], outs=[cc2_out])
        stats2 = persist.tile([128, 5], f32)
        nc.sync.dma_start(out=stats2, in_=cc2_out)
        finalize(2, stats2[:, 0:1], stats2[:, 1:2], stats2[:, 2:3],
                 stats2[:, 3:4], stats2[:, 4:5],
                 scl[:, 4:5], scl[:, 5:6])

        # ---------------- q gate + output ----------------
        qpre = wk_pool.tile([128, PTS], f16, tag="qpre")
        nc.vector.tensor_add(qpre, m_cm[:, 2 * PTS:3 * PTS],
                             c_cm[:, 2 * PTS:3 * PTS])
        q_sb = persist.tile([O, PTS], f32)
        nc.scalar.activation(out=q_sb, in_=qpre, func=AF.Tanh,
                             scale=scl[:, 4:5], bias=scl[:, 5:6])
        # delta = z*(q-h), f16 (h re-added on host in f32). int8 was tried
        # and rejected: its quantization floor is ~1.2e-2 rel err here.
        dfin = persist.tile([O, PTS], f32)
        nc.vector.tensor_sub(dfin, q_sb, hmy_sb)
        nc.vector.tensor_mul(dfin, dfin, z_sb)
        dfin16 = persist.tile([O, PTS], f16)
        nc.scalar.activation(out=dfin16, in_=dfin, func=AF.Copy)
        nc.sync.dma_start(out=out_io, in_=dfin16)

    nc.compile()
    return nc


def _prep_inputs(h, x, pc, Wz, bz, Wr, br, Wq, bq):
    """Host-side slicing/stacking -> per-core in_maps (minimal bytes)."""
    f32, f16 = np.float32, np.float16
    h, x, pc = np.asarray(h), np.asarray(x), np.asarray(pc)
    Wz, Wr, Wq = np.asarray(Wz), np.asarray(Wr), np.asarray(Wq)
    bz, br, bq = np.asarray(bz), np.asarray(br), np.asarray(bq)
    # stacked transposed weights [387, 384]; q's h-block removed (added in ph2)
    Wq_m = np.array(Wq, copy=True)
    Wq_m[:, 3:3 + H] = 0.0
    WT = np.concatenate([Wz.T, Wr.T, Wq_m.T], axis=1).astype(f32)  # [387, 384]
    # f16 weight block [128, 1280] = [wt0 | wt1 | wt2 | wqh]
    W16 = np.concatenate([WT[3:131], WT[131:259], WT[259:387],
                          Wq[:, 3:3 + H].T], axis=1).astype(f16)
    bcol = np.stack([bz, br, bq], axis=1).astype(f32)              # [128, 3]
    # wtg rows + stacked-bias row, appended to ppack cols PTS: of rows 0:4
    wb4 = np.concatenate([WT[0:3],
                          np.concatenate([bz, br, bq])[None, :]],
                         axis=0).astype(f32)                       # [4, 384]

    h16 = h.astype(f16)
    x16 = x.astype(f16)

    sq = (pc * pc).sum(axis=1, keepdims=True)                      # [B, 1, S]
    pca_full = np.concatenate([pc, sq], axis=1).astype(f32)        # [B, 4, S]

    in_maps = []
    for core in range(NCORES):
        b = core // GROUP
        r0 = (core % GROUP) * PTS
        # per-tile interleave [h|x0|x1]: [128, 8, 3, 128] -> [128, 3072],
        # then the core's 1/8 weight-table slice appended
        hx = np.empty((128, HXC + WSH), f16)
        hx[:, :HXC] = np.stack(
            [h16[b][:, r0:r0 + PTS].reshape(H, ST, 128),
             x16[b][:128, r0:r0 + PTS].reshape(128, ST, 128),
             x16[b][128:, r0:r0 + PTS].reshape(128, ST, 128)],
            axis=2).reshape(128, HXC)
        hx[:, HXC:] = W16[:, core * WSH:(core + 1) * WSH]
        ppack = np.zeros((8, PTS + 3 * O), f32)
        ppack[0:4, 0:PTS] = pca_full[b][:, r0:r0 + PTS]
        ppack[4:7, 0:PTS] = -2.0 * pc[b][:, r0:r0 + PTS]
        ppack[7, 0:PTS] = 1.0
        ppack[0:4, PTS:] = wb4
        in_maps.append({
            "hx16": hx,
            "ppack": ppack,
            "bcol": bcol,
        })
    return in_maps


def _enable_jax_compile_cache():
    """Persistent XLA compilation cache: run_bass_kernel_spmd re-lowers and
    re-compiles the NEFF-wrapped executable on every call (fresh jit closure
    per call); with the cache enabled, repeat calls hit the on-disk entry and
    skip ~0.2s of backend compile + BIR verify per call."""
    import os
    import tempfile
    import jax
    cache_dir = os.path.join(tempfile.gettempdir(), "jax_comp_cache_kernel")
    try:
        jax.config.update("jax_compilation_cache_dir", cache_dir)
        jax.config.update("jax_persistent_cache_min_compile_time_secs", 0.0)
        jax.config.update("jax_persistent_cache_min_entry_size_bytes", 0)
    except Exception:
        pass  # older jax without these flags: run uncached


def kernel(h, x, pc, Wz, bz, Wr, br, Wq, bq):
    from concourse.bass_utils import run_bass_kernel_spmd
    if "nc" not in _CACHE:
        _enable_jax_compile_cache()
        _CACHE["nc"] = _build_program()
    nc = _CACHE["nc"]
    h = np.asarray(h, dtype=np.float32)
    in_maps = _prep_inputs(h, x, pc, Wz, bz, Wr, br, Wq, bq)
    res = run_bass_kernel_spmd(nc, in_maps, core_ids=list(range(NCORES)))
    _CACHE["last_results"] = res
    out = np.zeros((B, H, S), np.float32)
    for core in range(NCORES):
        b = core // GROUP
        r0 = (core % GROUP) * PTS
        delta = res.results[core]["out"].astype(np.float32)
        out[b][:, r0:r0 + PTS] = h[b][:, r0:r0 + PTS] + delta
    return out


# revision 48
# speedup vs baseline: 1.1905x; 1.0150x over previous
"""Trainium2 Bass kernel for point-cloud GRU (kNN set-conv gates, InstanceNorm).

Wall time through the axon tunnel is dominated by host<->device transfer
and per-call dispatch, not device compute, so the design minimizes moved
bytes and per-call work:
  - 4 cores per batch (B=2), each owning a 1024-point shard of S=4096.
  - Each core receives ONLY its own f16 h/x shard (tile-interleaved),
    a 1/8 slice of the f16 weight tables, and tiny f32 packs; ~0.9MB/core
    vs ~7.3MB/core for replicated-f32 inputs.
  - The per-point projection tables (set-conv linearization, below) are
    computed per-shard from the core's own data, then AllGathered (f16)
    within the batch group -- cheaper and shorter-chained than gathering
    raw h/x and computing full tables on every core.
  - Output is the f16 GRU delta z*(q-h) only; full-precision h is added
    back on the host.
  - A persistent XLA compilation cache is enabled: run_bass_kernel_spmd
    re-lowers+re-compiles its jit closure every call otherwise (~0.2s).

Device algorithm:
  - kNN (k=4): PE computes score[i,j] = |x_j|^2 - 2 x_i.x_j for own rows,
    DVE max8+max_index on negated scores -> 4 smallest (self included).
  - Set-conv is linearized: y[s,k,o] = w[idx[s,k], o] + c[o, s] where
    w[n,o] = W_feat.f[n] + W_xyz.xyz[n] (per-point projection table) and
    c[o,s] = b[o] - W_xyz.xyz[s].  Table rows gathered by neighbor index
    (SWDGE indirect DMA) from the AllGathered [S, 384] f16 table.
  - InstanceNorm stats over (S,k) per (b,o) from algebraic identities:
      sum y   = A + k*Cs,   A  = sum_s t[s],  t = sum_k w[idx[s,k]]
      sum y^2 = B2 + 2*X + k*C2,  B2 = sum_s sum_k w^2,  X = sum_s c.t
    A/B2/X via PE ones-matmuls; Cs/C2 via ScalarE accum; partials
    AllReduced across the 4-core batch group (tiny).
  - max_k commutes with the (monotonic) normalization: out uses m = max_k w.
  - Phase 2 (q gate) needs r at neighbor points: per-shard q-table tiles
    Wq_h.(r*h) are computed locally then AllGathered (f16, [S, 128]).
"""

import numpy as np

B, S, H, D = 2, 4096, 128, 256
O = 128
K = 4
NCORES = 8
GROUP = 4              # cores per batch
PTS = S // GROUP       # points per core
NT = S // 128          # 32 table M-tiles
ST = PTS // 128        # 8 own s-tiles
EPS = 1e-5
NK = float(S * K)
WCOLS = 3 * (3 * O) + O        # wt0|wt1|wt2|wqh stacked -> 1280 f16 cols
WSH = WCOLS // 8               # 1280 / 8 = 160
HXC = 3 * PTS          # 3072 f16 cols: per-tile [h|x0|x1] interleave

_CACHE = {}


def _build_program():
    from concourse import bass, bacc, mybir, tile
    from concourse.masks import make_identity

    dt = mybir.dt
    f32, f16, u32 = dt.float32, dt.float16, dt.uint32
    AF = mybir.ActivationFunctionType
    ALU = mybir.AluOpType

    nc = bacc.Bacc("TRN2", target_bir_lowering=False, debug=False,
                   enable_asserts=False, num_devices=NCORES)

    # ---------------- I/O ----------------
    # hx16 cols: 0:HXC tile-interleaved [h|x0|x1] shard, HXC:HXC+WSH weight slice
    hx16 = nc.dram_tensor("hx16", [128, HXC + WSH], f16,
                          kind="ExternalInput").ap()
    # ppack rows: 0:4 pca_my ([pc;sq] own shard), 4:8 pct ([-2pc;ones]);
    # cols PTS:PTS+3*O of rows 0:4 hold wtg (3 rows) + brow (row 3)
    ppack = nc.dram_tensor("ppack", [8, PTS + 3 * O], f32,
                           kind="ExternalInput").ap()
    bcol = nc.dram_tensor("bcol", [128, 3], f32, kind="ExternalInput").ap()
    out_io = nc.dram_tensor("out", [O, PTS], f16, kind="ExternalOutput").ap()

    # ---------------- internal DRAM ----------------
    ag0_in = nc.dram_tensor("ag0_in", [4, PTS], f32, kind="Internal").ap()
    ag0_out = nc.dram_tensor("ag0_out", [GROUP, 4, PTS], f32,
                             kind="Internal").ap()
    ag2_in = nc.dram_tensor("ag2_in", [128, WSH], f16, kind="Internal").ap()
    ag2_out = nc.dram_tensor("ag2_out", [NCORES, 128, WSH], f16,
                             kind="Internal").ap()
    tb1own = nc.dram_tensor("tb1own", [PTS, 3 * O], f16, kind="Internal").ap()
    tb1 = nc.dram_tensor("tb1", [S, 3 * O], f16, kind="Internal").ap()
    tb2own = nc.dram_tensor("tb2own", [PTS, O], f16, kind="Internal").ap()
    tb2 = nc.dram_tensor("tb2", [S, O], f16, kind="Internal").ap()
    cc1_in = nc.dram_tensor("cc1_in", [128, 10], f32, kind="Internal").ap()
    cc1_out = nc.dram_tensor("cc1_out", [128, 10], f32, kind="Internal").ap()
    cc2_in = nc.dram_tensor("cc2_in", [128, 5], f32, kind="Internal").ap()
    cc2_out = nc.dram_tensor("cc2_out", [128, 5], f32, kind="Internal").ap()

    RG = [[0, 1, 2, 3], [4, 5, 6, 7]]
    RG8 = [[0, 1, 2, 3, 4, 5, 6, 7]]

    from contextlib import ExitStack
    ctx = ExitStack()
    with tile.TileContext(nc) as tc, ctx:
        persist = ctx.enter_context(tc.tile_pool(name="persist", bufs=1))
        gst_pool = ctx.enter_context(tc.tile_pool(name="gst", bufs=1))
        sc_pool = ctx.enter_context(tc.tile_pool(name="scores", bufs=2))
        wk_pool = ctx.enter_context(tc.tile_pool(name="work", bufs=2))
        ps_pool = ctx.enter_context(tc.tile_pool(name="ps", bufs=6, space="PSUM"))
        px_pool = ctx.enter_context(tc.tile_pool(name="psX", bufs=1, space="PSUM"))

        def psum(shape, tag="ps", dtp=None):
            return ps_pool.tile(shape, dtp or f32, tag=tag, name=tag)

        # ---- persistent SBUF ----
        hx_sb = persist.tile([128, HXC], f16)           # own h/x shard
        wsh_sb = persist.tile([128, WSH], f16)
        wtall_sb = persist.tile([128, WCOLS], f16)      # gathered weights
        pca_sb = persist.tile([4, S], f32)
        pcam_sb = persist.tile([4, PTS], f32)
        pct_sb = persist.tile([4, PTS], f32)
        wtg_sb = persist.tile([3, 3 * O], f32)
        bcol_sb = persist.tile([128, 3], f32)
        brow_sb = persist.tile([1, 3 * O], f32)
        hmy_sb = persist.tile([H, PTS], f32)            # own h, f32, contiguous
        idx_sb = persist.tile([128, 8 * ST], u32)
        ones16 = persist.tile([128, 1], f16)
        onesK = persist.tile([1, 128], f32)
        ident = persist.tile([128, 128], f16)
        b_bc = persist.tile([128, 3 * O], f16)
        c_cm = persist.tile([128, 3 * PTS], f16)        # c channel-major, per gate
        csum = persist.tile([128, 12], f32)             # Cs/C2 halves per gate
        m_cm = persist.tile([128, 3 * PTS], f16)        # gathered-max, channel-major
        c_pm = [persist.tile([128, 3 * O], f16, tag=f"c_pm{i}", name=f"c_pm{i}")
                for i in range(ST)]
        stats_sb = persist.tile([128, 10], f32)
        scl = persist.tile([128, 8], f32)               # istd/nbias per gate
        z_sb = persist.tile([O, PTS], f32)
        r_sb = persist.tile([O, PTS], f32)

        stats_ps = px_pool.tile([128, 96], f32)         # PE stat columns

        # ---- stage + launch the two input AllGathers first ----
        nc.sync.dma_start(out=pcam_sb, in_=ppack[0:4, 0:PTS])
        nc.sync.dma_start(out=ag0_in, in_=pcam_sb)
        nc.gpsimd.collective_compute("AllGather", mybir.AluOpType.bypass,
                                     replica_groups=RG,
                                     ins=[ag0_in], outs=[ag0_out])
        nc.sync.dma_start(out=hx_sb, in_=hx16[:, 0:HXC])
        nc.sync.dma_start(out=wsh_sb, in_=hx16[:, HXC:HXC + WSH])
        nc.sync.dma_start(out=ag2_in, in_=wsh_sb)
        nc.gpsimd.collective_compute("AllGather", mybir.AluOpType.bypass,
                                     replica_groups=RG8,
                                     ins=[ag2_in], outs=[ag2_out])

        # full pca from the gathered shards
        for rk in range(GROUP):
            nc.sync.dma_start(out=pca_sb[:, rk * PTS:(rk + 1) * PTS],
                              in_=ag0_out[rk])
        nc.sync.dma_start(out=pct_sb, in_=ppack[4:8, 0:PTS])

        nc.sync.dma_start(out=wtg_sb, in_=ppack[0:3, PTS:PTS + 3 * O])
        nc.sync.dma_start(out=bcol_sb, in_=bcol)
        nc.sync.dma_start(out=brow_sb, in_=ppack[3:4, PTS:PTS + 3 * O])

        nc.vector.memset(ones16, 1.0)
        nc.vector.memset(onesK, 1.0)
        make_identity(nc, ident[:])

        # own h -> contiguous f32 [H, PTS] (tile-deinterleave from hx_sb)
        for t in range(ST):
            nc.scalar.activation(out=hmy_sb[:, t * 128:(t + 1) * 128],
                                 in_=hx_sb[:, t * 384:t * 384 + 128],
                                 func=AF.Copy)

        # gathered weight slices -> one contiguous SBUF block
        for k in range(NCORES):
            nc.sync.dma_start(out=wtall_sb[:, k * WSH:(k + 1) * WSH],
                              in_=ag2_out[k])
        wt0_sb = wtall_sb[:, 0:3 * O]
        wt1_sb = wtall_sb[:, 3 * O:6 * O]
        wt2_sb = wtall_sb[:, 6 * O:9 * O]
        wqh_sb = wtall_sb[:, 9 * O:9 * O + O]

        # b broadcast down partitions (point-major bias): ones^T @ brow
        psb = psum([128, 3 * O])
        nc.tensor.matmul(out=psb, lhsT=onesK, rhs=brow_sb, start=True, stop=True)
        nc.scalar.activation(out=b_bc, in_=psb, func=AF.Copy)

        # ---- scores + top-4 (overlaps the AllGathers) ----
        def emit_score(st):
            srow = sc_pool.tile([128, S], f32, tag="srow", name="srow")
            for ch in range(8):
                ps = psum([128, 512])
                nc.tensor.matmul(out=ps,
                                 lhsT=pct_sb[:, st * 128:(st + 1) * 128],
                                 rhs=pca_sb[:, ch * 512:(ch + 1) * 512],
                                 start=True, stop=True)
                # negate so max8 finds the smallest distances
                nc.scalar.activation(out=srow[:, ch * 512:(ch + 1) * 512],
                                     in_=ps, func=AF.Copy, scale=-1.0)
            mx = wk_pool.tile([128, 8], f32, tag="mx8", name="mx8")
            nc.vector.max(out=mx, in_=srow)
            nc.vector.max_index(out=idx_sb[:, st * 8:st * 8 + 8],
                                in_max=mx, in_values=srow)

        # w table (z | r | q-static): OWN tiles only, from own SBUF shard;
        # the full [S, 384] table is then AllGathered (f16, 768KB in).
        def emit_own_table(st):
            pst = psum([128, 3 * O])
            nc.tensor.matmul(out=pst, lhsT=hx_sb[:, st * 384:st * 384 + 128],
                             rhs=wt0_sb, start=True, stop=False)
            nc.tensor.matmul(out=pst,
                             lhsT=hx_sb[:, st * 384 + 128:st * 384 + 256],
                             rhs=wt1_sb, start=False, stop=False)
            nc.tensor.matmul(out=pst,
                             lhsT=hx_sb[:, st * 384 + 256:st * 384 + 384],
                             rhs=wt2_sb, start=False, stop=False)
            nc.tensor.matmul(out=pst,
                             lhsT=pcam_sb[0:3, st * 128:(st + 1) * 128],
                             rhs=wtg_sb, start=False, stop=True)
            tb_sb = wk_pool.tile([128, 3 * O], f16, tag="tb_sb", name="tb_sb")
            nc.scalar.activation(out=tb_sb, in_=pst, func=AF.Copy)
            nc.sync.dma_start(out=tb1own[st * 128:(st + 1) * 128, :], in_=tb_sb)

        # scores first in the PE queue: they gate on ag0 (group AllGather)
        # which completes no later than ag2, avoiding head-of-line blocking
        for st in range(ST):
            emit_score(st)
        for st in range(ST):
            emit_own_table(st)
        nc.gpsimd.collective_compute("AllGather", mybir.AluOpType.bypass,
                                     replica_groups=RG,
                                     ins=[tb1own], outs=[tb1])

        # ---------------- c tiles ----------------
        # channel-major: c[o, s] = b[o] - v[o, s];  Cs/C2 via ScalarE accum.
        for g in range(3):
            for hh in range(2):
                psv = psum([128, 512])
                nc.tensor.matmul(out=psv,
                                 lhsT=wtg_sb[:, g * O:(g + 1) * O],
                                 rhs=pcam_sb[0:3, hh * 512:(hh + 1) * 512],
                                 start=True, stop=True)
                cs = slice(g * PTS + hh * 512, g * PTS + (hh + 1) * 512)
                nc.scalar.activation(out=c_cm[:, cs], in_=psv, func=AF.Identity,
                                     bias=bcol_sb[:, g:g + 1], scale=-1.0,
                                     accum_out=csum[:, 4 * g + hh:4 * g + hh + 1])
                scr = wk_pool.tile([128, 512], f16, tag="c2scr")
                nc.scalar.activation(out=scr, in_=psv, func=AF.Square,
                                     bias=bcol_sb[:, g:g + 1], scale=-1.0,
                                     accum_out=csum[:, 4 * g + 2 + hh:4 * g + 3 + hh])

        # point-major c tiles (for the X statistic)
        for st in range(ST):
            psv2 = psum([128, 3 * O])
            nc.tensor.matmul(out=psv2,
                             lhsT=pcam_sb[0:3, st * 128:(st + 1) * 128],
                             rhs=wtg_sb, start=True, stop=True)
            nc.scalar.activation(out=c_pm[st], in_=psv2, func=AF.Copy, scale=-1.0)
            nc.vector.tensor_add(c_pm[st], c_pm[st], b_bc)

        # ---------------- phase-1 gathers + folds (z, r) ----------------
        gtiles = [[gst_pool.tile([128, 3 * O], f16, tag=f"g{st}_{j}",
                              name=f"g{st}_{j}") for j in range(K)]
                  for st in range(ST)]
        for st in range(ST):
            g0, g1, g2, g3 = gtiles[st]
            for j in range(K):
                nc.gpsimd.indirect_dma_start(
                    out=gtiles[st][j][:], out_offset=None, in_=tb1[:, :],
                    in_offset=bass.IndirectOffsetOnAxis(
                        ap=idx_sb[:, st * 8 + j:st * 8 + j + 1], axis=0))
            zr = slice(0, 2 * O)
            t = wk_pool.tile([128, 2 * O], f16, tag="t_zr")
            nc.vector.tensor_add(t, g0[:, zr], g1[:, zr])
            nc.vector.tensor_add(t, t, g2[:, zr])
            nc.vector.tensor_add(t, t, g3[:, zr])
            m = wk_pool.tile([128, 2 * O], f16, tag="m_zr")
            nc.vector.tensor_max(m, g0[:, zr], g1[:, zr])
            nc.vector.tensor_max(m, m, g2[:, zr])
            nc.vector.tensor_max(m, m, g3[:, zr])
            t2 = wk_pool.tile([128, 2 * O], f16, tag="t2_zr")
            sq = wk_pool.tile([128, 2 * O], f16, tag="sq_zr")
            nc.scalar.activation(out=t2, in_=g0[:, zr], func=AF.Square)
            nc.scalar.activation(out=sq, in_=g1[:, zr], func=AF.Square)
            nc.vector.tensor_add(t2, t2, sq)
            nc.scalar.activation(out=sq, in_=g2[:, zr], func=AF.Square)
            nc.vector.tensor_add(t2, t2, sq)
            nc.scalar.activation(out=sq, in_=g3[:, zr], func=AF.Square)
            nc.vector.tensor_add(t2, t2, sq)
            ct = wk_pool.tile([128, 2 * O], f16, tag="ct_zr")
            nc.vector.tensor_mul(ct, c_pm[st][:, zr], t)
            for qi, srct in ((0, t), (2, t2), (4, ct)):
                for gx in range(2):
                    col = (qi + gx) * 8 + st
                    nc.tensor.matmul(out=stats_ps[:, col:col + 1],
                                     lhsT=srct[:, gx * O:(gx + 1) * O],
                                     rhs=ones16, start=True, stop=True)
            # transpose m -> channel-major
            for gx in range(2):
                ptr = psum([128, 128], dtp=f16)
                nc.tensor.transpose(out=ptr, in_=m[:, gx * O:(gx + 1) * O],
                                    identity=ident)
                nc.scalar.activation(
                    out=m_cm[:, gx * PTS + st * 128:gx * PTS + (st + 1) * 128],
                    in_=ptr, func=AF.Copy)

        # ---------------- stats AllReduce #1 (z, r) ----------------
        ccp = persist.tile([128, 10], f32)
        # cols: A B2 X Cs C2 per gate
        for gx in range(2):
            nc.vector.tensor_reduce(out=ccp[:, 5 * gx + 0:5 * gx + 1],
                                    in_=stats_ps[:, (0 + gx) * 8:(0 + gx) * 8 + 8],
                                    axis=mybir.AxisListType.X, op=ALU.add)
            nc.vector.tensor_reduce(out=ccp[:, 5 * gx + 1:5 * gx + 2],
                                    in_=stats_ps[:, (2 + gx) * 8:(2 + gx) * 8 + 8],
                                    axis=mybir.AxisListType.X, op=ALU.add)
            nc.vector.tensor_reduce(out=ccp[:, 5 * gx + 2:5 * gx + 3],
                                    in_=stats_ps[:, (4 + gx) * 8:(4 + gx) * 8 + 8],
                                    axis=mybir.AxisListType.X, op=ALU.add)
            nc.vector.tensor_add(ccp[:, 5 * gx + 3:5 * gx + 4],
                                 csum[:, 4 * gx:4 * gx + 1],
                                 csum[:, 4 * gx + 1:4 * gx + 2])
            nc.vector.tensor_add(ccp[:, 5 * gx + 4:5 * gx + 5],
                                 csum[:, 4 * gx + 2:4 * gx + 3],
                                 csum[:, 4 * gx + 3:4 * gx + 4])
        nc.sync.dma_start(out=cc1_in, in_=ccp)
        nc.gpsimd.collective_compute("AllReduce", mybir.AluOpType.add,
                                     replica_groups=RG,
                                     ins=[cc1_in], outs=[cc1_out])
        nc.sync.dma_start(out=stats_sb, in_=cc1_out)

        # ---------------- finalize gate scale/bias ----------------
        def finalize(gx, A, B2, X, Cs, C2, o_istd, o_nbias):
            w1 = wk_pool.tile([128, 1], f32, tag="fw1")
            w2 = wk_pool.tile([128, 1], f32, tag="fw2")
            w3 = wk_pool.tile([128, 1], f32, tag="fw3")
            # mu = (A + 4*Cs)/NK
            nc.vector.tensor_scalar(w1, Cs, 4.0, None, op0=ALU.mult)
            nc.vector.tensor_add(w1, w1, A)
            nc.vector.tensor_scalar(w1, w1, 1.0 / NK, None, op0=ALU.mult)
            # Ey2 = (B2 + 2X + 4*C2)/NK
            nc.vector.tensor_scalar(w2, X, 2.0, None, op0=ALU.mult)
            nc.vector.tensor_add(w2, w2, B2)
            nc.vector.tensor_scalar(w3, C2, 4.0, None, op0=ALU.mult)
            nc.vector.tensor_add(w2, w2, w3)
            nc.vector.tensor_scalar(w2, w2, 1.0 / NK, None, op0=ALU.mult)
            # var = Ey2 - mu^2 ; istd = 1/sqrt(var+eps); nbias = -mu*istd
            nc.vector.tensor_mul(w3, w1, w1)
            nc.vector.tensor_sub(w2, w2, w3)
            nc.vector.tensor_scalar_add(w2, w2, EPS)
            nc.scalar.activation(out=w2, in_=w2, func=AF.Sqrt)
            nc.vector.reciprocal(o_istd, w2)
            nc.vector.tensor_mul(o_nbias, w1, o_istd)
            nc.vector.tensor_scalar(o_nbias, o_nbias, -1.0, None, op0=ALU.mult)

        for gx in range(2):
            c0 = 5 * gx
            finalize(gx,
                     stats_sb[:, c0:c0 + 1], stats_sb[:, c0 + 1:c0 + 2],
                     stats_sb[:, c0 + 2:c0 + 3], stats_sb[:, c0 + 3:c0 + 4],
                     stats_sb[:, c0 + 4:c0 + 5],
                     scl[:, 2 * gx:2 * gx + 1], scl[:, 2 * gx + 1:2 * gx + 2])

        # ---------------- z, r gates ----------------
        for gx, dst in ((0, z_sb), (1, r_sb)):
            pre = wk_pool.tile([128, PTS], f16, tag="pre")
            nc.vector.tensor_add(pre, m_cm[:, gx * PTS:(gx + 1) * PTS],
                                 c_cm[:, gx * PTS:(gx + 1) * PTS])
            nc.scalar.activation(out=dst, in_=pre, func=AF.Sigmoid,
                                 scale=scl[:, 2 * gx:2 * gx + 1],
                                 bias=scl[:, 2 * gx + 1:2 * gx + 2])

        # ---------------- q table (dynamic part): own tiles + AllGather ----
        rh = wk_pool.tile([H, PTS], f16, tag="rh")
        nc.vector.tensor_mul(rh, r_sb, hmy_sb)
        for st in range(ST):
            ps2 = psum([128, O])
            nc.tensor.matmul(out=ps2, lhsT=rh[:, st * 128:(st + 1) * 128],
                             rhs=wqh_sb, start=True, stop=True)
            tq_sb = wk_pool.tile([128, O], f16, tag="tq_sb")
            nc.scalar.activation(out=tq_sb, in_=ps2, func=AF.Copy)
            nc.sync.dma_start(out=tb2own[st * 128:(st + 1) * 128, :], in_=tq_sb)
        nc.gpsimd.collective_compute("AllGather", mybir.AluOpType.bypass,
                                     replica_groups=RG,
                                     ins=[tb2own], outs=[tb2])

        # ---------------- phase-2 gathers + folds (q) ----------------
        qs = slice(2 * O, 3 * O)
        for st in range(ST):
            gq = [wk_pool.tile([128, O], f16, tag=f"gq{j}", name=f"gq{j}")
                  for j in range(K)]
            for j in range(K):
                nc.gpsimd.indirect_dma_start(
                    out=gq[j][:], out_offset=None, in_=tb2[:, :],
                    in_offset=bass.IndirectOffsetOnAxis(
                        ap=idx_sb[:, st * 8 + j:st * 8 + j + 1], axis=0))
                nc.vector.tensor_add(gq[j], gq[j], gtiles[st][j][:, qs])
            t = wk_pool.tile([128, O], f16, tag="t_q")
            nc.vector.tensor_add(t, gq[0], gq[1])
            nc.vector.tensor_add(t, t, gq[2])
            nc.vector.tensor_add(t, t, gq[3])
            m = wk_pool.tile([128, O], f16, tag="m_q")
            nc.vector.tensor_max(m, gq[0], gq[1])
            nc.vector.tensor_max(m, m, gq[2])
            nc.vector.tensor_max(m, m, gq[3])
            t2 = wk_pool.tile([128, O], f16, tag="t2_q")
            sq = wk_pool.tile([128, O], f16, tag="sq_q")
            nc.scalar.activation(out=t2, in_=gq[0], func=AF.Square)
            nc.scalar.activation(out=sq, in_=gq[1], func=AF.Square)
            nc.vector.tensor_add(t2, t2, sq)
            nc.scalar.activation(out=sq, in_=gq[2], func=AF.Square)
            nc.vector.tensor_add(t2, t2, sq)
            nc.scalar.activation(out=sq, in_=gq[3], func=AF.Square)
            nc.vector.tensor_add(t2, t2, sq)
            ct = wk_pool.tile([128, O], f16, tag="ct_q")
            nc.vector.tensor_mul(ct, c_pm[st][:, qs], t)
            for qi, srct in ((6, t), (7, t2), (8, ct)):
                col = qi * 8 + st
                nc.tensor.matmul(out=stats_ps[:, col:col + 1], lhsT=srct,
                                 rhs=ones16, start=True, stop=True)
            ptr = psum([128, 128], dtp=f16)
            nc.tensor.transpose(out=ptr, in_=m, identity=ident)
            nc.scalar.activation(
                out=m_cm[:, 2 * PTS + st * 128:2 * PTS + (st + 1) * 128],
                in_=ptr, func=AF.Copy)

        # ---------------- stats AllReduce #2 (q) ----------------
        ccq = persist.tile([128, 5], f32)
        nc.vector.tensor_reduce(out=ccq[:, 0:1], in_=stats_ps[:, 48:56],
                                axis=mybir.AxisListType.X, op=ALU.add)
        nc.vector.tensor_reduce(out=ccq[:, 1:2], in_=stats_ps[:, 56:64],
                                axis=mybir.AxisListType.X, op=ALU.add)
        nc.vector.tensor_reduce(out=ccq[:, 2:3], in_=stats_ps[:, 64:72],
                                axis=mybir.AxisListType.X, op=ALU.add)
        nc.vector.tensor_add(ccq[:, 3:4], csum[:, 8:9], csum[:, 9:10])
        nc.vector.tensor_add(ccq[:, 4:5], csum[:, 10:11], csum[:, 11:12])
        nc.sync.dma_start(out=cc2_in, in_=ccq)
        nc.gpsimd.collective_compute("AllReduce", mybir.AluOpType.add,
                                     replica_groups=RG,
                                     ins=[cc2_in], outs=[cc2_out])
        stats2 = persist.tile([128, 5], f32)
        nc.sync.dma_start(out=stats2, in_=cc2_out)
        finalize(2, stats2[:, 0:1], stats2[:, 1:2], stats2[:, 2:3],
                 stats2[:, 3:4], stats2[:, 4:5],
                 scl[:, 4:5], scl[:, 5:6])

        # ---------------- q gate + output ----------------
        qpre = wk_pool.tile([128, PTS], f16, tag="qpre")
        nc.vector.tensor_add(qpre, m_cm[:, 2 * PTS:3 * PTS],
                             c_cm[:, 2 * PTS:3 * PTS])
        q_sb = persist.tile([O, PTS], f32)
        nc.scalar.activation(out=q_sb, in_=qpre, func=AF.Tanh,
                             scale=scl[:, 4:5], bias=scl[:, 5:6])
        # delta = z*(q-h), f16 (h re-added on host in f32). int8 was tried
        # and rejected: its quantization floor is ~1.2e-2 rel err here.
        dfin = persist.tile([O, PTS], f32)
        nc.vector.tensor_sub(dfin, q_sb, hmy_sb)
        nc.vector.tensor_mul(dfin, dfin, z_sb)
        dfin16 = persist.tile([O, PTS], f16)
        nc.scalar.activation(out=dfin16, in_=dfin, func=AF.Copy)
        nc.sync.dma_start(out=out_io, in_=dfin16)

    nc.compile()
    return nc


def _prep_inputs(h, x, pc, Wz, bz, Wr, br, Wq, bq):
    """Host-side slicing/stacking -> per-core in_maps (minimal bytes)."""
    f32, f16 = np.float32, np.float16
    h, x, pc = np.asarray(h), np.asarray(x), np.asarray(pc)
    Wz, Wr, Wq = np.asarray(Wz), np.asarray(Wr), np.asarray(Wq)
    bz, br, bq = np.asarray(bz), np.asarray(br), np.asarray(bq)
    # stacked transposed weights [387, 384]; q's h-block removed (added in ph2)
    Wq_m = np.array(Wq, copy=True)
    Wq_m[:, 3:3 + H] = 0.0
    WT = np.concatenate([Wz.T, Wr.T, Wq_m.T], axis=1).astype(f32)  # [387, 384]
    # f16 weight block [128, 1280] = [wt0 | wt1 | wt2 | wqh]
    W16 = np.concatenate([WT[3:131], WT[131:259], WT[259:387],
                          Wq[:, 3:3 + H].T], axis=1).astype(f16)
    bcol = np.stack([bz, br, bq], axis=1).astype(f32)              # [128, 3]
    # wtg rows + stacked-bias row, appended to ppack cols PTS: of rows 0:4
    wb4 = np.concatenate([WT[0:3],
                          np.concatenate([bz, br, bq])[None, :]],
                         axis=0).astype(f32)                       # [4, 384]

    sq = (pc * pc).sum(axis=1, keepdims=True)                      # [B, 1, S]
    pca_full = np.concatenate([pc, sq], axis=1).astype(f32)        # [B, 4, S]

    def _core_map(core):
        # astype/stack release the GIL on arrays this size -> thread-friendly
        b = core // GROUP
        r0 = (core % GROUP) * PTS
        # per-tile interleave [h|x0|x1]: [128, 8, 3, 128] -> [128, 3072],
        # then the core's 1/8 weight-table slice appended
        hx = np.empty((128, HXC + WSH), f16)
        hx[:, :HXC] = np.stack(
            [h[b][:, r0:r0 + PTS].astype(f16).reshape(H, ST, 128),
             x[b][:128, r0:r0 + PTS].astype(f16).reshape(128, ST, 128),
             x[b][128:, r0:r0 + PTS].astype(f16).reshape(128, ST, 128)],
            axis=2).reshape(128, HXC)
        hx[:, HXC:] = W16[:, core * WSH:(core + 1) * WSH]
        ppack = np.zeros((8, PTS + 3 * O), f32)
        ppack[0:4, 0:PTS] = pca_full[b][:, r0:r0 + PTS]
        ppack[4:7, 0:PTS] = -2.0 * pc[b][:, r0:r0 + PTS]
        ppack[7, 0:PTS] = 1.0
        ppack[0:4, PTS:] = wb4
        return {"hx16": hx, "ppack": ppack, "bcol": bcol}

    from concurrent.futures import ThreadPoolExecutor
    if "pool" not in _CACHE:
        _CACHE["pool"] = ThreadPoolExecutor(max_workers=4)
    return list(_CACHE["pool"].map(_core_map, range(NCORES)))


def _enable_jax_compile_cache():
    """Persistent XLA compilation cache: run_bass_kernel_spmd re-lowers and
    re-compiles the NEFF-wrapped executable on every call (fresh jit closure
    per call); with the cache enabled, repeat calls hit the on-disk entry and
    skip ~0.2s of backend compile + BIR verify per call."""
    import os
    import tempfile
    import jax
    cache_dir = os.path.join(tempfile.gettempdir(), "jax_comp_cache_kernel")
    try:
        jax.config.update("jax_compilation_cache_dir", cache_dir)
        jax.config.update("jax_persistent_cache_min_compile_time_secs", 0.0)
        jax.config.update("jax_persistent_cache_min_entry_size_bytes", 0)
    except Exception:
        pass  # older jax without these flags: run uncached


def kernel(h, x, pc, Wz, bz, Wr, br, Wq, bq):
    from concourse.bass_utils import run_bass_kernel_spmd
    if "nc" not in _CACHE:
        _enable_jax_compile_cache()
        _CACHE["nc"] = _build_program()
    nc = _CACHE["nc"]
    h = np.asarray(h, dtype=np.float32)
    in_maps = _prep_inputs(h, x, pc, Wz, bz, Wr, br, Wq, bq)
    res = run_bass_kernel_spmd(nc, in_maps, core_ids=list(range(NCORES)))
    _CACHE["last_results"] = res
    delta = np.stack([res.results[c]["out"] for c in range(NCORES)])
    delta = delta.reshape(B, GROUP, H, PTS).transpose(0, 2, 1, 3)
    return h + delta.reshape(B, H, S)


# revision 51
# speedup vs baseline: 1.2141x; 1.0198x over previous
"""Trainium2 Bass kernel for point-cloud GRU (kNN set-conv gates, InstanceNorm).

Wall time through the axon tunnel is dominated by host<->device transfer
and per-call dispatch, not device compute, so the design minimizes moved
bytes and per-call work:
  - 4 cores per batch (B=2), each owning a 1024-point shard of S=4096.
  - Each core receives ONLY its own f16 h/x shard (tile-interleaved),
    a 1/8 slice of the f16 weight tables, and tiny f32 packs; ~0.9MB/core
    vs ~7.3MB/core for replicated-f32 inputs.
  - The per-point projection tables (set-conv linearization, below) are
    computed per-shard from the core's own data, then AllGathered (f16)
    within the batch group -- cheaper and shorter-chained than gathering
    raw h/x and computing full tables on every core.
  - Output is the f16 GRU delta z*(q-h) only; full-precision h is added
    back on the host.
  - A persistent XLA compilation cache is enabled: run_bass_kernel_spmd
    re-lowers+re-compiles its jit closure every call otherwise (~0.2s).

Device algorithm:
  - kNN (k=4): PE computes score[i,j] = |x_j|^2 - 2 x_i.x_j for own rows,
    DVE max8+max_index on negated scores -> 4 smallest (self included).
  - Set-conv is linearized: y[s,k,o] = w[idx[s,k], o] + c[o, s] where
    w[n,o] = W_feat.f[n] + W_xyz.xyz[n] (per-point projection table) and
    c[o,s] = b[o] - W_xyz.xyz[s].  Table rows gathered by neighbor index
    (SWDGE indirect DMA) from the AllGathered [S, 384] f16 table.
  - InstanceNorm stats over (S,k) per (b,o) from algebraic identities:
      sum y   = A + k*Cs,   A  = sum_s t[s],  t = sum_k w[idx[s,k]]
      sum y^2 = B2 + 2*X + k*C2,  B2 = sum_s sum_k w^2,  X = sum_s c.t
    A/B2/X via PE ones-matmuls; Cs/C2 via ScalarE accum; partials
    AllReduced across the 4-core batch group (tiny).
  - max_k commutes with the (monotonic) normalization: out uses m = max_k w.
  - Phase 2 (q gate) needs r at neighbor points: per-shard q-table tiles
    Wq_h.(r*h) are computed locally then AllGathered (f16, [S, 128]).
"""

import numpy as np

B, S, H, D = 2, 4096, 128, 256
O = 128
K = 4
NCORES = 8
GROUP = 4              # cores per batch
PTS = S // GROUP       # points per core
NT = S // 128          # 32 table M-tiles
ST = PTS // 128        # 8 own s-tiles
EPS = 1e-5
NK = float(S * K)
WCOLS = 3 * (3 * O) + O        # wt0|wt1|wt2|wqh stacked -> 1280 f16 cols
WSH = WCOLS // 8               # 1280 / 8 = 160
HXC = 3 * PTS          # 3072 f16 cols: per-tile [h|x0|x1] interleave

_CACHE = {}


def _build_program():
    from concourse import bass, bacc, mybir, tile
    from concourse.masks import make_identity

    dt = mybir.dt
    f32, f16, u32 = dt.float32, dt.float16, dt.uint32
    AF = mybir.ActivationFunctionType
    ALU = mybir.AluOpType

    nc = bacc.Bacc("TRN2", target_bir_lowering=False, debug=False,
                   enable_asserts=False, num_devices=NCORES)

    # ---------------- I/O ----------------
    # hx16 cols: 0:HXC tile-interleaved [h|x0|x1] shard, HXC:HXC+WSH weight slice
    hx16 = nc.dram_tensor("hx16", [128, HXC + WSH], f16,
                          kind="ExternalInput").ap()
    # ppack rows: 0:4 pca_my ([pc;sq] own shard), 4:8 pct ([-2pc;ones]);
    # cols PTS:PTS+3*O of rows 0:3 hold wtg. The conv biases bz/br/bq are
    # dropped entirely: InstanceNorm subtracts the per-channel mean, so a
    # constant channel bias cancels exactly (verified vs the reference).
    ppack = nc.dram_tensor("ppack", [8, PTS + 3 * O], f32,
                           kind="ExternalInput").ap()
    out_io = nc.dram_tensor("out", [O, PTS], f16, kind="ExternalOutput").ap()

    # ---------------- internal DRAM ----------------
    ag0_in = nc.dram_tensor("ag0_in", [4, PTS], f32, kind="Internal").ap()
    ag0_out = nc.dram_tensor("ag0_out", [GROUP, 4, PTS], f32,
                             kind="Internal").ap()
    ag2_in = nc.dram_tensor("ag2_in", [128, WSH], f16, kind="Internal").ap()
    ag2_out = nc.dram_tensor("ag2_out", [NCORES, 128, WSH], f16,
                             kind="Internal").ap()
    tb1own = nc.dram_tensor("tb1own", [PTS, 3 * O], f16, kind="Internal").ap()
    tb1 = nc.dram_tensor("tb1", [S, 3 * O], f16, kind="Internal").ap()
    tb2own = nc.dram_tensor("tb2own", [PTS, O], f16, kind="Internal").ap()
    tb2 = nc.dram_tensor("tb2", [S, O], f16, kind="Internal").ap()
    cc1_in = nc.dram_tensor("cc1_in", [128, 10], f32, kind="Internal").ap()
    cc1_out = nc.dram_tensor("cc1_out", [128, 10], f32, kind="Internal").ap()
    cc2_in = nc.dram_tensor("cc2_in", [128, 5], f32, kind="Internal").ap()
    cc2_out = nc.dram_tensor("cc2_out", [128, 5], f32, kind="Internal").ap()

    RG = [[0, 1, 2, 3], [4, 5, 6, 7]]
    RG8 = [[0, 1, 2, 3, 4, 5, 6, 7]]

    from contextlib import ExitStack
    ctx = ExitStack()
    with tile.TileContext(nc) as tc, ctx:
        persist = ctx.enter_context(tc.tile_pool(name="persist", bufs=1))
        gst_pool = ctx.enter_context(tc.tile_pool(name="gst", bufs=1))
        sc_pool = ctx.enter_context(tc.tile_pool(name="scores", bufs=2))
        wk_pool = ctx.enter_context(tc.tile_pool(name="work", bufs=2))
        ps_pool = ctx.enter_context(tc.tile_pool(name="ps", bufs=6, space="PSUM"))
        px_pool = ctx.enter_context(tc.tile_pool(name="psX", bufs=1, space="PSUM"))

        def psum(shape, tag="ps", dtp=None):
            return ps_pool.tile(shape, dtp or f32, tag=tag, name=tag)

        # ---- persistent SBUF ----
        hx_sb = persist.tile([128, HXC], f16)           # own h/x shard
        wsh_sb = persist.tile([128, WSH], f16)
        wtall_sb = persist.tile([128, WCOLS], f16)      # gathered weights
        pca_sb = persist.tile([4, S], f32)
        pcam_sb = persist.tile([4, PTS], f32)
        pct_sb = persist.tile([4, PTS], f32)
        wtg_sb = persist.tile([3, 3 * O], f32)
        hmy_sb = persist.tile([H, PTS], f32)            # own h, f32, contiguous
        idx_sb = persist.tile([128, 8 * ST], u32)
        ones16 = persist.tile([128, 1], f16)
        ident = persist.tile([128, 128], f16)
        c_cm = persist.tile([128, 3 * PTS], f16)        # c channel-major, per gate
        csum = persist.tile([128, 12], f32)             # Cs/C2 halves per gate
        m_cm = persist.tile([128, 3 * PTS], f16)        # gathered-max, channel-major
        c_pm = [persist.tile([128, 3 * O], f16, tag=f"c_pm{i}", name=f"c_pm{i}")
                for i in range(ST)]
        stats_sb = persist.tile([128, 10], f32)
        scl = persist.tile([128, 8], f32)               # istd/nbias per gate
        z_sb = persist.tile([O, PTS], f32)
        r_sb = persist.tile([O, PTS], f32)

        stats_ps = px_pool.tile([128, 96], f32)         # PE stat columns

        # ---- stage + launch the two input AllGathers first ----
        nc.sync.dma_start(out=pcam_sb, in_=ppack[0:4, 0:PTS])
        nc.sync.dma_start(out=ag0_in, in_=pcam_sb)
        nc.gpsimd.collective_compute("AllGather", mybir.AluOpType.bypass,
                                     replica_groups=RG,
                                     ins=[ag0_in], outs=[ag0_out])
        nc.sync.dma_start(out=hx_sb, in_=hx16[:, 0:HXC])
        nc.sync.dma_start(out=wsh_sb, in_=hx16[:, HXC:HXC + WSH])
        nc.sync.dma_start(out=ag2_in, in_=wsh_sb)
        nc.gpsimd.collective_compute("AllGather", mybir.AluOpType.bypass,
                                     replica_groups=RG8,
                                     ins=[ag2_in], outs=[ag2_out])

        # full pca from the gathered shards
        for rk in range(GROUP):
            nc.sync.dma_start(out=pca_sb[:, rk * PTS:(rk + 1) * PTS],
                              in_=ag0_out[rk])
        nc.sync.dma_start(out=pct_sb, in_=ppack[4:8, 0:PTS])

        nc.sync.dma_start(out=wtg_sb, in_=ppack[0:3, PTS:PTS + 3 * O])

        nc.vector.memset(ones16, 1.0)
        make_identity(nc, ident[:])

        # own h -> contiguous f32 [H, PTS] (tile-deinterleave from hx_sb)
        for t in range(ST):
            nc.scalar.activation(out=hmy_sb[:, t * 128:(t + 1) * 128],
                                 in_=hx_sb[:, t * 384:t * 384 + 128],
                                 func=AF.Copy)

        # gathered weight slices -> one contiguous SBUF block
        for k in range(NCORES):
            nc.sync.dma_start(out=wtall_sb[:, k * WSH:(k + 1) * WSH],
                              in_=ag2_out[k])
        wt0_sb = wtall_sb[:, 0:3 * O]
        wt1_sb = wtall_sb[:, 3 * O:6 * O]
        wt2_sb = wtall_sb[:, 6 * O:9 * O]
        wqh_sb = wtall_sb[:, 9 * O:9 * O + O]

        # ---- scores + top-4 (overlaps the AllGathers) ----
        def emit_score(st):
            srow = sc_pool.tile([128, S], f32, tag="srow", name="srow")
            for ch in range(8):
                ps = psum([128, 512])
                nc.tensor.matmul(out=ps,
                                 lhsT=pct_sb[:, st * 128:(st + 1) * 128],
                                 rhs=pca_sb[:, ch * 512:(ch + 1) * 512],
                                 start=True, stop=True)
                # negate so max8 finds the smallest distances
                nc.scalar.activation(out=srow[:, ch * 512:(ch + 1) * 512],
                                     in_=ps, func=AF.Copy, scale=-1.0)
            mx = wk_pool.tile([128, 8], f32, tag="mx8", name="mx8")
            nc.vector.max(out=mx, in_=srow)
            nc.vector.max_index(out=idx_sb[:, st * 8:st * 8 + 8],
                                in_max=mx, in_values=srow)

        # w table (z | r | q-static): OWN tiles only, from own SBUF shard;
        # the full [S, 384] table is then AllGathered (f16, 768KB in).
        def emit_own_table(st):
            pst = psum([128, 3 * O])
            nc.tensor.matmul(out=pst, lhsT=hx_sb[:, st * 384:st * 384 + 128],
                             rhs=wt0_sb, start=True, stop=False)
            nc.tensor.matmul(out=pst,
                             lhsT=hx_sb[:, st * 384 + 128:st * 384 + 256],
                             rhs=wt1_sb, start=False, stop=False)
            nc.tensor.matmul(out=pst,
                             lhsT=hx_sb[:, st * 384 + 256:st * 384 + 384],
                             rhs=wt2_sb, start=False, stop=False)
            nc.tensor.matmul(out=pst,
                             lhsT=pcam_sb[0:3, st * 128:(st + 1) * 128],
                             rhs=wtg_sb, start=False, stop=True)
            tb_sb = wk_pool.tile([128, 3 * O], f16, tag="tb_sb", name="tb_sb")
            nc.scalar.activation(out=tb_sb, in_=pst, func=AF.Copy)
            nc.sync.dma_start(out=tb1own[st * 128:(st + 1) * 128, :], in_=tb_sb)

        # scores first in the PE queue: they gate on ag0 (group AllGather)
        # which completes no later than ag2, avoiding head-of-line blocking
        for st in range(ST):
            emit_score(st)
        for st in range(ST):
            emit_own_table(st)
        nc.gpsimd.collective_compute("AllGather", mybir.AluOpType.bypass,
                                     replica_groups=RG,
                                     ins=[tb1own], outs=[tb1])

        # ---------------- c tiles ----------------
        # channel-major: c[o, s] = b[o] - v[o, s];  Cs/C2 via ScalarE accum.
        for g in range(3):
            for hh in range(2):
                psv = psum([128, 512])
                nc.tensor.matmul(out=psv,
                                 lhsT=wtg_sb[:, g * O:(g + 1) * O],
                                 rhs=pcam_sb[0:3, hh * 512:(hh + 1) * 512],
                                 start=True, stop=True)
                cs = slice(g * PTS + hh * 512, g * PTS + (hh + 1) * 512)
                nc.scalar.activation(out=c_cm[:, cs], in_=psv, func=AF.Copy,
                                     scale=-1.0,
                                     accum_out=csum[:, 4 * g + hh:4 * g + hh + 1])
                scr = wk_pool.tile([128, 512], f16, tag="c2scr")
                nc.scalar.activation(out=scr, in_=psv, func=AF.Square,
                                     scale=-1.0,
                                     accum_out=csum[:, 4 * g + 2 + hh:4 * g + 3 + hh])

        # point-major c tiles (for the X statistic)
        for st in range(ST):
            psv2 = psum([128, 3 * O])
            nc.tensor.matmul(out=psv2,
                             lhsT=pcam_sb[0:3, st * 128:(st + 1) * 128],
                             rhs=wtg_sb, start=True, stop=True)
            nc.scalar.activation(out=c_pm[st], in_=psv2, func=AF.Copy, scale=-1.0)

        # ---------------- phase-1 gathers + folds (z, r) ----------------
        gtiles = [[gst_pool.tile([128, 3 * O], f16, tag=f"g{st}_{j}",
                              name=f"g{st}_{j}") for j in range(K)]
                  for st in range(ST)]
        for st in range(ST):
            g0, g1, g2, g3 = gtiles[st]
            for j in range(K):
                nc.gpsimd.indirect_dma_start(
                    out=gtiles[st][j][:], out_offset=None, in_=tb1[:, :],
                    in_offset=bass.IndirectOffsetOnAxis(
                        ap=idx_sb[:, st * 8 + j:st * 8 + j + 1], axis=0))
            zr = slice(0, 2 * O)
            t = wk_pool.tile([128, 2 * O], f16, tag="t_zr")
            nc.vector.tensor_add(t, g0[:, zr], g1[:, zr])
            nc.vector.tensor_add(t, t, g2[:, zr])
            nc.vector.tensor_add(t, t, g3[:, zr])
            m = wk_pool.tile([128, 2 * O], f16, tag="m_zr")
            nc.vector.tensor_max(m, g0[:, zr], g1[:, zr])
            nc.vector.tensor_max(m, m, g2[:, zr])
            nc.vector.tensor_max(m, m, g3[:, zr])
            t2 = wk_pool.tile([128, 2 * O], f16, tag="t2_zr")
            sq = wk_pool.tile([128, 2 * O], f16, tag="sq_zr")
            nc.scalar.activation(out=t2, in_=g0[:, zr], func=AF.Square)
            nc.scalar.activation(out=sq, in_=g1[:, zr], func=AF.Square)
            nc.vector.tensor_add(t2, t2, sq)
            nc.scalar.activation(out=sq, in_=g2[:, zr], func=AF.Square)
            nc.vector.tensor_add(t2, t2, sq)
            nc.scalar.activation(out=sq, in_=g3[:, zr], func=AF.Square)
            nc.vector.tensor_add(t2, t2, sq)
            ct = wk_pool.tile([128, 2 * O], f16, tag="ct_zr")
            nc.vector.tensor_mul(ct, c_pm[st][:, zr], t)
            for qi, srct in ((0, t), (2, t2), (4, ct)):
                for gx in range(2):
                    col = (qi + gx) * 8 + st
                    nc.tensor.matmul(out=stats_ps[:, col:col + 1],
                                     lhsT=srct[:, gx * O:(gx + 1) * O],
                                     rhs=ones16, start=True, stop=True)
            # transpose m -> channel-major
            for gx in range(2):
                ptr = psum([128, 128], dtp=f16)
                nc.tensor.transpose(out=ptr, in_=m[:, gx * O:(gx + 1) * O],
                                    identity=ident)
                nc.scalar.activation(
                    out=m_cm[:, gx * PTS + st * 128:gx * PTS + (st + 1) * 128],
                    in_=ptr, func=AF.Copy)

        # ---------------- stats AllReduce #1 (z, r) ----------------
        ccp = persist.tile([128, 10], f32)
        # cols: A B2 X Cs C2 per gate
        for gx in range(2):
            nc.vector.tensor_reduce(out=ccp[:, 5 * gx + 0:5 * gx + 1],
                                    in_=stats_ps[:, (0 + gx) * 8:(0 + gx) * 8 + 8],
                                    axis=mybir.AxisListType.X, op=ALU.add)
            nc.vector.tensor_reduce(out=ccp[:, 5 * gx + 1:5 * gx + 2],
                                    in_=stats_ps[:, (2 + gx) * 8:(2 + gx) * 8 + 8],
                                    axis=mybir.AxisListType.X, op=ALU.add)
            nc.vector.tensor_reduce(out=ccp[:, 5 * gx + 2:5 * gx + 3],
                                    in_=stats_ps[:, (4 + gx) * 8:(4 + gx) * 8 + 8],
                                    axis=mybir.AxisListType.X, op=ALU.add)
            nc.vector.tensor_add(ccp[:, 5 * gx + 3:5 * gx + 4],
                                 csum[:, 4 * gx:4 * gx + 1],
                                 csum[:, 4 * gx + 1:4 * gx + 2])
            nc.vector.tensor_add(ccp[:, 5 * gx + 4:5 * gx + 5],
                                 csum[:, 4 * gx + 2:4 * gx + 3],
                                 csum[:, 4 * gx + 3:4 * gx + 4])
        nc.sync.dma_start(out=cc1_in, in_=ccp)
        nc.gpsimd.collective_compute("AllReduce", mybir.AluOpType.add,
                                     replica_groups=RG,
                                     ins=[cc1_in], outs=[cc1_out])
        nc.sync.dma_start(out=stats_sb, in_=cc1_out)

        # ---------------- finalize gate scale/bias ----------------
        def finalize(gx, A, B2, X, Cs, C2, o_istd, o_nbias):
            w1 = wk_pool.tile([128, 1], f32, tag="fw1")
            w2 = wk_pool.tile([128, 1], f32, tag="fw2")
            w3 = wk_pool.tile([128, 1], f32, tag="fw3")
            # mu = (A + 4*Cs)/NK
            nc.vector.tensor_scalar(w1, Cs, 4.0, None, op0=ALU.mult)
            nc.vector.tensor_add(w1, w1, A)
            nc.vector.tensor_scalar(w1, w1, 1.0 / NK, None, op0=ALU.mult)
            # Ey2 = (B2 + 2X + 4*C2)/NK
            nc.vector.tensor_scalar(w2, X, 2.0, None, op0=ALU.mult)
            nc.vector.tensor_add(w2, w2, B2)
            nc.vector.tensor_scalar(w3, C2, 4.0, None, op0=ALU.mult)
            nc.vector.tensor_add(w2, w2, w3)
            nc.vector.tensor_scalar(w2, w2, 1.0 / NK, None, op0=ALU.mult)
            # var = Ey2 - mu^2 ; istd = 1/sqrt(var+eps); nbias = -mu*istd
            nc.vector.tensor_mul(w3, w1, w1)
            nc.vector.tensor_sub(w2, w2, w3)
            nc.vector.tensor_scalar_add(w2, w2, EPS)
            nc.scalar.activation(out=w2, in_=w2, func=AF.Sqrt)
            nc.vector.reciprocal(o_istd, w2)
            nc.vector.tensor_mul(o_nbias, w1, o_istd)
            nc.vector.tensor_scalar(o_nbias, o_nbias, -1.0, None, op0=ALU.mult)

        for gx in range(2):
            c0 = 5 * gx
            finalize(gx,
                     stats_sb[:, c0:c0 + 1], stats_sb[:, c0 + 1:c0 + 2],
                     stats_sb[:, c0 + 2:c0 + 3], stats_sb[:, c0 + 3:c0 + 4],
                     stats_sb[:, c0 + 4:c0 + 5],
                     scl[:, 2 * gx:2 * gx + 1], scl[:, 2 * gx + 1:2 * gx + 2])

        # ---------------- z, r gates ----------------
        for gx, dst in ((0, z_sb), (1, r_sb)):
            pre = wk_pool.tile([128, PTS], f16, tag="pre")
            nc.vector.tensor_add(pre, m_cm[:, gx * PTS:(gx + 1) * PTS],
                                 c_cm[:, gx * PTS:(gx + 1) * PTS])
            nc.scalar.activation(out=dst, in_=pre, func=AF.Sigmoid,
                                 scale=scl[:, 2 * gx:2 * gx + 1],
                                 bias=scl[:, 2 * gx + 1:2 * gx + 2])

        # ---------------- q table (dynamic part): own tiles + AllGather ----
        rh = wk_pool.tile([H, PTS], f16, tag="rh")
        nc.vector.tensor_mul(rh, r_sb, hmy_sb)
        for st in range(ST):
            ps2 = psum([128, O])
            nc.tensor.matmul(out=ps2, lhsT=rh[:, st * 128:(st + 1) * 128],
                             rhs=wqh_sb, start=True, stop=True)
            tq_sb = wk_pool.tile([128, O], f16, tag="tq_sb")
            nc.scalar.activation(out=tq_sb, in_=ps2, func=AF.Copy)
            nc.sync.dma_start(out=tb2own[st * 128:(st + 1) * 128, :], in_=tq_sb)
        nc.gpsimd.collective_compute("AllGather", mybir.AluOpType.bypass,
                                     replica_groups=RG,
                                     ins=[tb2own], outs=[tb2])

        # ---------------- phase-2 gathers + folds (q) ----------------
        qs = slice(2 * O, 3 * O)
        for st in range(ST):
            gq = [wk_pool.tile([128, O], f16, tag=f"gq{j}", name=f"gq{j}")
                  for j in range(K)]
            for j in range(K):
                nc.gpsimd.indirect_dma_start(
                    out=gq[j][:], out_offset=None, in_=tb2[:, :],
                    in_offset=bass.IndirectOffsetOnAxis(
                        ap=idx_sb[:, st * 8 + j:st * 8 + j + 1], axis=0))
                nc.vector.tensor_add(gq[j], gq[j], gtiles[st][j][:, qs])
            t = wk_pool.tile([128, O], f16, tag="t_q")
            nc.vector.tensor_add(t, gq[0], gq[1])
            nc.vector.tensor_add(t, t, gq[2])
            nc.vector.tensor_add(t, t, gq[3])
            m = wk_pool.tile([128, O], f16, tag="m_q")
            nc.vector.tensor_max(m, gq[0], gq[1])
            nc.vector.tensor_max(m, m, gq[2])
            nc.vector.tensor_max(m, m, gq[3])
            t2 = wk_pool.tile([128, O], f16, tag="t2_q")
            sq = wk_pool.tile([128, O], f16, tag="sq_q")
            nc.scalar.activation(out=t2, in_=gq[0], func=AF.Square)
            nc.scalar.activation(out=sq, in_=gq[1], func=AF.Square)
            nc.vector.tensor_add(t2, t2, sq)
            nc.scalar.activation(out=sq, in_=gq[2], func=AF.Square)
            nc.vector.tensor_add(t2, t2, sq)
            nc.scalar.activation(out=sq, in_=gq[3], func=AF.Square)
            nc.vector.tensor_add(t2, t2, sq)
            ct = wk_pool.tile([128, O], f16, tag="ct_q")
            nc.vector.tensor_mul(ct, c_pm[st][:, qs], t)
            for qi, srct in ((6, t), (7, t2), (8, ct)):
                col = qi * 8 + st
                nc.tensor.matmul(out=stats_ps[:, col:col + 1], lhsT=srct,
                                 rhs=ones16, start=True, stop=True)
            ptr = psum([128, 128], dtp=f16)
            nc.tensor.transpose(out=ptr, in_=m, identity=ident)
            nc.scalar.activation(
                out=m_cm[:, 2 * PTS + st * 128:2 * PTS + (st + 1) * 128],
                in_=ptr, func=AF.Copy)

        # ---------------- stats AllReduce #2 (q) ----------------
        ccq = persist.tile([128, 5], f32)
        nc.vector.tensor_reduce(out=ccq[:, 0:1], in_=stats_ps[:, 48:56],
                                axis=mybir.AxisListType.X, op=ALU.add)
        nc.vector.tensor_reduce(out=ccq[:, 1:2], in_=stats_ps[:, 56:64],
                                axis=mybir.AxisListType.X, op=ALU.add)
        nc.vector.tensor_reduce(out=ccq[:, 2:3], in_=stats_ps[:, 64:72],
                                axis=mybir.AxisListType.X, op=ALU.add)
        nc.vector.tensor_add(ccq[:, 3:4], csum[:, 8:9], csum[:, 9:10])
        nc.vector.tensor_add(ccq[:, 4:5], csum[:, 10:11], csum[:, 11:12])
        nc.sync.dma_start(out=cc2_in, in_=ccq)
        nc.gpsimd.collective_compute("AllReduce", mybir.AluOpType.add,
                                     replica_groups=RG,
                                     ins=[cc2_in], outs=[cc2_out])
        stats2 = persist.tile([128, 5], f32)
        nc.sync.dma_start(out=stats2, in_=cc2_out)
        finalize(2, stats2[:, 0:1], stats2[:, 1:2], stats2[:, 2:3],
                 stats2[:, 3:4], stats2[:, 4:5],
                 scl[:, 4:5], scl[:, 5:6])

        # ---------------- q gate + output ----------------
        qpre = wk_pool.tile([128, PTS], f16, tag="qpre")
        nc.vector.tensor_add(qpre, m_cm[:, 2 * PTS:3 * PTS],
                             c_cm[:, 2 * PTS:3 * PTS])
        q_sb = persist.tile([O, PTS], f32)
        nc.scalar.activation(out=q_sb, in_=qpre, func=AF.Tanh,
                             scale=scl[:, 4:5], bias=scl[:, 5:6])
        # delta = z*(q-h), f16 (h re-added on host in f32). int8 was tried
        # and rejected: its quantization floor is ~1.2e-2 rel err here.
        dfin = persist.tile([O, PTS], f32)
        nc.vector.tensor_sub(dfin, q_sb, hmy_sb)
        nc.vector.tensor_mul(dfin, dfin, z_sb)
        dfin16 = persist.tile([O, PTS], f16)
        nc.scalar.activation(out=dfin16, in_=dfin, func=AF.Copy)
        nc.sync.dma_start(out=out_io, in_=dfin16)

    nc.compile()
    return nc


def _prep_inputs(h, x, pc, Wz, bz, Wr, br, Wq, bq):
    """Host-side slicing/stacking -> per-core in_maps (minimal bytes)."""
    f32, f16 = np.float32, np.float16
    h, x, pc = np.asarray(h), np.asarray(x), np.asarray(pc)
    Wz, Wr, Wq = np.asarray(Wz), np.asarray(Wr), np.asarray(Wq)
    bz, br, bq = np.asarray(bz), np.asarray(br), np.asarray(bq)
    # stacked transposed weights [387, 384]; q's h-block removed (added in ph2)
    Wq_m = np.array(Wq, copy=True)
    Wq_m[:, 3:3 + H] = 0.0
    WT = np.concatenate([Wz.T, Wr.T, Wq_m.T], axis=1).astype(f32)  # [387, 384]
    # f16 weight block [128, 1280] = [wt0 | wt1 | wt2 | wqh]
    W16 = np.concatenate([WT[3:131], WT[131:259], WT[259:387],
                          Wq[:, 3:3 + H].T], axis=1).astype(f16)
    # biases bz/br/bq are not sent: InstanceNorm cancels constant channel
    # biases exactly (mean subtraction), verified against the reference
    wtg3 = np.ascontiguousarray(WT[0:3]).astype(f32)               # [3, 384]

    sq = (pc * pc).sum(axis=1, keepdims=True)                      # [B, 1, S]
    pca_full = np.concatenate([pc, sq], axis=1).astype(f32)        # [B, 4, S]

    def _core_map(core):
        # astype/stack release the GIL on arrays this size -> thread-friendly
        b = core // GROUP
        r0 = (core % GROUP) * PTS
        # per-tile interleave [h|x0|x1]: [128, 8, 3, 128] -> [128, 3072],
        # then the core's 1/8 weight-table slice appended
        hx = np.empty((128, HXC + WSH), f16)
        hx[:, :HXC] = np.stack(
            [h[b][:, r0:r0 + PTS].astype(f16).reshape(H, ST, 128),
             x[b][:128, r0:r0 + PTS].astype(f16).reshape(128, ST, 128),
             x[b][128:, r0:r0 + PTS].astype(f16).reshape(128, ST, 128)],
            axis=2).reshape(128, HXC)
        hx[:, HXC:] = W16[:, core * WSH:(core + 1) * WSH]
        ppack = np.zeros((8, PTS + 3 * O), f32)
        ppack[0:4, 0:PTS] = pca_full[b][:, r0:r0 + PTS]
        ppack[4:7, 0:PTS] = -2.0 * pc[b][:, r0:r0 + PTS]
        ppack[7, 0:PTS] = 1.0
        ppack[0:3, PTS:] = wtg3
        return {"hx16": hx, "ppack": ppack}

    from concurrent.futures import ThreadPoolExecutor
    if "pool" not in _CACHE:
        _CACHE["pool"] = ThreadPoolExecutor(max_workers=4)
    return list(_CACHE["pool"].map(_core_map, range(NCORES)))


def _enable_jax_compile_cache():
    """Persistent XLA compilation cache: run_bass_kernel_spmd re-lowers and
    re-compiles the NEFF-wrapped executable on every call (fresh jit closure
    per call); with the cache enabled, repeat calls hit the on-disk entry and
    skip ~0.2s of backend compile + BIR verify per call."""
    import os
    import tempfile
    import jax
    cache_dir = os.path.join(tempfile.gettempdir(), "jax_comp_cache_kernel")
    try:
        jax.config.update("jax_compilation_cache_dir", cache_dir)
        jax.config.update("jax_persistent_cache_min_compile_time_secs", 0.0)
        jax.config.update("jax_persistent_cache_min_entry_size_bytes", 0)
    except Exception:
        pass  # older jax without these flags: run uncached


def kernel(h, x, pc, Wz, bz, Wr, br, Wq, bq):
    from concourse.bass_utils import run_bass_kernel_spmd
    if "nc" not in _CACHE:
        _enable_jax_compile_cache()
        _CACHE["nc"] = _build_program()
    nc = _CACHE["nc"]
    h = np.asarray(h, dtype=np.float32)
    in_maps = _prep_inputs(h, x, pc, Wz, bz, Wr, br, Wq, bq)
    res = run_bass_kernel_spmd(nc, in_maps, core_ids=list(range(NCORES)))
    _CACHE["last_results"] = res
    delta = np.stack([res.results[c]["out"] for c in range(NCORES)])
    delta = delta.reshape(B, GROUP, H, PTS).transpose(0, 2, 1, 3)
    return h + delta.reshape(B, H, S)
